# revision 2
# baseline (speedup 1.0000x reference)
"""Trainium2 Bass kernel for the per-pixel locally-connected MLP (dense_mlp).

Reference computation (per batch b, pixel (h,w)):
    x0 = coor (2-vector, shared by all pixels)
    h1 = relu(W0 @ x0)        W0 = weight[b, 0:32].reshape(16, 2)   per pixel
    h2 = relu(W1 @ h1)        W1 = weight[b, 32:288].reshape(16,16) per pixel
    y  = W2 @ h2 + bias       W2 = weight[b, 288:336].reshape(3,16), bias = weight[b,336]
Output: [4, 3, 256, 256] float32.

Sharding: 8 cores, core k handles batch k//2, rows (k%2)*128:(k%2+1)*128
=> per-core weight shard [337, 32768] (channels x pixels), no cross-core comm.

On-chip layout: channels on SBUF partitions, pixels on the free axis.
Per-pixel matvecs = elementwise multiplies (VectorE) + partition-axis
reductions (TensorE matmuls against small 0/1 selection matrices built on the
host; `coor` is folded into the first matmul's stationary matrix).
"""

import sys

for _p in ("/opt/trn_rl_repo", "/root/.axon_site/_ro/trn_rl_repo"):
    if _p not in sys.path:
        sys.path.append(_p)

import numpy as np

import concourse.bass as bass
import concourse.tile as tile
from concourse import bacc, mybir
from concourse.bass_utils import run_bass_kernel_spmd

# ---------------------------------------------------------------- constants
B, H, W = 4, 256, 256
N_CH = 337            # 32 (L0) + 256 (L1) + 48 (L2) + 1 (bias)
N_CORES = 8
PIX = (B * H * W) // N_CORES  # 32768 pixels per core
F = 512               # pixels per compute chunk (one PSUM bank of fp32)
N_CHUNKS = PIX // F

FP32 = mybir.dt.float32


def _const_mats(coor: np.ndarray) -> dict[str, np.ndarray]:
    """Small stationary matrices for the TensorE reductions."""
    cx, cy = float(coor[0]), float(coor[1])
    s0 = np.zeros((32, 16), np.float32)       # h1pre = S0.T @ w[0:32]
    for i in range(16):
        s0[2 * i, i] = cx
        s0[2 * i + 1, i] = cy
    r8 = np.zeros((16, 128), np.float32)      # h1rep[m] = h1[m % 16]
    for m in range(128):
        r8[m % 16, m] = 1.0
    m1a = np.zeros((128, 16), np.float32)     # h2pre[j] += sum_i prodA[16j+i]
    m1b = np.zeros((128, 16), np.float32)
    for k in range(128):
        m1a[k, k // 16] = 1.0
        m1b[k, 8 + k // 16] = 1.0
    r3 = np.zeros((16, 48), np.float32)       # h2rep[m] = h2[m % 16]
    for m in range(48):
        r3[m % 16, m] = 1.0
    m2 = np.zeros((48, 3), np.float32)        # y[j] = sum_i prodC[16j+i]
    for k in range(48):
        m2[k, k // 16] = 1.0
    ones1 = np.ones((1, 3), np.float32)       # y[j] += bias
    return {"s0": s0, "r8": r8, "m1a": m1a, "m1b": m1b, "r3": r3, "m2": m2,
            "ones1": ones1}


def build_nc(repeat: int = 1):
    """Build the per-core Bass program. `repeat` re-runs the whole kernel
    body sequentially (used only for differential HW timing)."""
    nc = bacc.Bacc(None, target_bir_lowering=False)

    w = nc.declare_dram_parameter("w", [N_CH, PIX], FP32, isOutput=False)
    out = nc.declare_dram_parameter("out", [3, PIX], FP32, isOutput=True)
    c_s0 = nc.declare_dram_parameter("s0", [32, 16], FP32, isOutput=False)
    c_r8 = nc.declare_dram_parameter("r8", [16, 128], FP32, isOutput=False)
    c_m1a = nc.declare_dram_parameter("m1a", [128, 16], FP32, isOutput=False)
    c_m1b = nc.declare_dram_parameter("m1b", [128, 16], FP32, isOutput=False)
    c_r3 = nc.declare_dram_parameter("r3", [16, 48], FP32, isOutput=False)
    c_m2 = nc.declare_dram_parameter("m2", [48, 3], FP32, isOutput=False)
    c_ones1 = nc.declare_dram_parameter("ones1", [1, 3], FP32, isOutput=False)

    with tile.TileContext(nc) as tc:
        with (
            tc.tile_pool(name="consts", bufs=1) as consts,
            tc.tile_pool(name="loads", bufs=3) as loads,
            tc.tile_pool(name="acts", bufs=3) as acts,
            tc.tile_pool(name="prods", bufs=2) as prods,
            tc.tile_pool(name="outs", bufs=2) as outs,
            tc.tile_pool(name="ps_pre", bufs=3, space="PSUM") as ps_pre,
            tc.tile_pool(name="ps_rep", bufs=2, space="PSUM") as ps_rep,
        ):
            s0 = consts.tile([32, 16], FP32)
            r8 = consts.tile([16, 128], FP32)
            m1a = consts.tile([128, 16], FP32)
            m1b = consts.tile([128, 16], FP32)
            r3 = consts.tile([16, 48], FP32)
            m2 = consts.tile([48, 3], FP32)
            ones1 = consts.tile([1, 3], FP32)
            for t, d in ((s0, c_s0), (r8, c_r8), (m1a, c_m1a), (m1b, c_m1b),
                         (r3, c_r3), (m2, c_m2), (ones1, c_ones1)):
                nc.sync.dma_start(out=t[:], in_=d[:])

            for _rep in range(repeat):
                for c in range(N_CHUNKS):
                    px = slice(c * F, (c + 1) * F)

                    t0 = loads.tile([32, F], FP32, tag="t0")
                    t1a = loads.tile([128, F], FP32, tag="t1a")
                    t1b = loads.tile([128, F], FP32, tag="t1b")
                    t2 = loads.tile([48, F], FP32, tag="t2")
                    tbias = loads.tile([1, F], FP32, tag="tbias")
                    nc.sync.dma_start(out=t0[:], in_=w[0:32, px])
                    nc.sync.dma_start(out=t1a[:], in_=w[32:160, px])
                    nc.sync.dma_start(out=t1b[:], in_=w[160:288, px])
                    nc.sync.dma_start(out=t2[:], in_=w[288:336, px])
                    nc.sync.dma_start(out=tbias[:], in_=w[336:337, px])

                    # ---- layer 0: h1 = relu(S0.T @ w0) --------------------
                    h1pre = ps_pre.tile([16, F], FP32, tag="pre")
                    nc.tensor.matmul(h1pre[:], s0[:], t0[:], start=True, stop=True)
                    h1 = acts.tile([16, F], FP32, tag="h1")
                    nc.scalar.activation(h1[:], h1pre[:],
                                         mybir.ActivationFunctionType.Relu)

                    # ---- layer 1: h2 = relu(sum_i w1[j,i] * h1[i]) --------
                    h1rep = ps_rep.tile([128, F], FP32, tag="rep")
                    nc.tensor.matmul(h1rep[:], r8[:], h1[:], start=True, stop=True)
                    prodA = prods.tile([128, F], FP32, tag="prodA")
                    prodB = prods.tile([128, F], FP32, tag="prodB")
                    nc.vector.tensor_mul(prodA[:], t1a[:], h1rep[:])
                    nc.vector.tensor_mul(prodB[:], t1b[:], h1rep[:])
                    h2pre = ps_pre.tile([16, F], FP32, tag="pre")
                    nc.tensor.matmul(h2pre[:], m1a[:], prodA[:], start=True, stop=False)
                    nc.tensor.matmul(h2pre[:], m1b[:], prodB[:], start=False, stop=True)
                    h2 = acts.tile([16, F], FP32, tag="h2")
                    nc.scalar.activation(h2[:], h2pre[:],
                                         mybir.ActivationFunctionType.Relu)

                    # ---- layer 2: y = sum_i w2[j,i] * h2[i] + bias --------
                    h2rep = ps_rep.tile([48, F], FP32, tag="rep")
                    nc.tensor.matmul(h2rep[:], r3[:], h2[:], start=True, stop=True)
                    prodC = prods.tile([48, F], FP32, tag="prodC")
                    nc.vector.tensor_mul(prodC[:], t2[:], h2rep[:])
                    y = ps_pre.tile([3, F], FP32, tag="pre")
                    nc.tensor.matmul(y[:], m2[:], prodC[:], start=True, stop=False)
                    nc.tensor.matmul(y[:], ones1[:], tbias[:], start=False,
                                     stop=True)

                    y_sb = outs.tile([3, F], FP32, tag="ysb")
                    nc.scalar.copy(y_sb[:], y[:])
                    nc.sync.dma_start(out=out[:, px], in_=y_sb[:])

    nc.compile()
    return nc


_NC_CACHE: dict[int, object] = {}


def _get_nc(repeat: int = 1):
    if repeat not in _NC_CACHE:
        _NC_CACHE[repeat] = build_nc(repeat)
    return _NC_CACHE[repeat]


def make_in_maps(weight: np.ndarray, coor: np.ndarray) -> list[dict]:
    mats = _const_mats(coor)
    in_maps = []
    for k in range(N_CORES):
        b, hh = k // 2, k % 2
        shard = np.ascontiguousarray(
            weight[b, :, hh * 128:(hh + 1) * 128, :].reshape(N_CH, PIX),
            dtype=np.float32)
        in_maps.append({"w": shard, **mats})
    return in_maps


def assemble_out(results: list[dict]) -> np.ndarray:
    out = np.empty((B, 3, H, W), np.float32)
    for k in range(N_CORES):
        b, hh = k // 2, k % 2
        out[b, :, hh * 128:(hh + 1) * 128, :] = results[k]["out"].reshape(3, 128, W)
    return out


def kernel(input: np.ndarray, weight: np.ndarray, coor: np.ndarray) -> np.ndarray:
    nc = _get_nc(1)
    in_maps = make_in_maps(np.asarray(weight), np.asarray(coor))
    res = run_bass_kernel_spmd(nc, in_maps, core_ids=list(range(N_CORES)))
    return assemble_out(res.results)


# revision 4
# speedup vs baseline: 56.3695x; 56.3695x over previous
"""Trainium2 Bass kernel for the per-pixel locally-connected MLP (dense_mlp).

Reference computation (per batch b, pixel (h,w)):
    x0 = coor (2-vector, shared by all pixels)
    h1 = relu(W0 @ x0)        W0 = weight[b, 0:32].reshape(16, 2)   per pixel
    h2 = relu(W1 @ h1)        W1 = weight[b, 32:288].reshape(16,16) per pixel
    y  = W2 @ h2 + bias       W2 = weight[b, 288:336].reshape(3,16), bias = weight[b,336]
Output: [4, 3, 256, 256] float32.

Sharding: 8 cores, core k handles batch k//2, rows (k%2)*128:(k%2+1)*128
=> per-core weight shard [337, 32768] (channels x pixels), no cross-core comm.

On-chip layout: channels on SBUF partitions, pixels on the free axis.
Per-pixel matvecs = elementwise multiplies (VectorE) + partition-axis
reductions (TensorE matmuls against small 0/1 selection matrices built on the
host; `coor` is folded into the first matmul's stationary matrix).
"""

import sys

for _p in ("/opt/trn_rl_repo", "/root/.axon_site/_ro/trn_rl_repo"):
    if _p not in sys.path:
        sys.path.append(_p)

import numpy as np

import concourse.bass as bass
import concourse.tile as tile
from concourse import bacc, mybir
from concourse.bass_utils import run_bass_kernel_spmd

# ---------------------------------------------------------------- constants
B, H, W = 4, 256, 256
N_CH = 337            # 32 (L0) + 256 (L1) + 48 (L2) + 1 (bias)
N_CORES = 8
PIX = (B * H * W) // N_CORES  # 32768 pixels per core
F = 512               # pixels per compute chunk (one PSUM bank of fp32)
N_CHUNKS = PIX // F

FP32 = mybir.dt.float32


def _const_mats(coor: np.ndarray) -> dict[str, np.ndarray]:
    """Small stationary matrices for the TensorE reductions."""
    cx, cy = float(coor[0]), float(coor[1])
    s0 = np.zeros((32, 16), np.float32)       # h1pre = S0.T @ w[0:32]
    for i in range(16):
        s0[2 * i, i] = cx
        s0[2 * i + 1, i] = cy
    r8 = np.zeros((16, 128), np.float32)      # h1rep[m] = h1[m % 16]
    for m in range(128):
        r8[m % 16, m] = 1.0
    m1a = np.zeros((128, 16), np.float32)     # h2pre[j] += sum_i prodA[16j+i]
    m1b = np.zeros((128, 16), np.float32)
    for k in range(128):
        m1a[k, k // 16] = 1.0
        m1b[k, 8 + k // 16] = 1.0
    r3 = np.zeros((16, 48), np.float32)       # h2rep[m] = h2[m % 16]
    for m in range(48):
        r3[m % 16, m] = 1.0
    m2 = np.zeros((48, 3), np.float32)        # y[j] = sum_i prodC[16j+i]
    for k in range(48):
        m2[k, k // 16] = 1.0
    ones1 = np.ones((1, 3), np.float32)       # y[j] += bias
    return {"s0": s0, "r8": r8, "m1a": m1a, "m1b": m1b, "r3": r3, "m2": m2,
            "ones1": ones1}


def build_nc(repeat: int = 1):
    """Build the per-core Bass program. `repeat` re-runs the whole kernel
    body sequentially (used only for differential HW timing)."""
    nc = bacc.Bacc(None, target_bir_lowering=False)

    w = nc.declare_dram_parameter("w", [N_CH, PIX], FP32, isOutput=False)
    out = nc.declare_dram_parameter("out", [3, PIX], FP32, isOutput=True)
    c_s0 = nc.declare_dram_parameter("s0", [32, 16], FP32, isOutput=False)
    c_r8 = nc.declare_dram_parameter("r8", [16, 128], FP32, isOutput=False)
    c_m1a = nc.declare_dram_parameter("m1a", [128, 16], FP32, isOutput=False)
    c_m1b = nc.declare_dram_parameter("m1b", [128, 16], FP32, isOutput=False)
    c_r3 = nc.declare_dram_parameter("r3", [16, 48], FP32, isOutput=False)
    c_m2 = nc.declare_dram_parameter("m2", [48, 3], FP32, isOutput=False)
    c_ones1 = nc.declare_dram_parameter("ones1", [1, 3], FP32, isOutput=False)

    with tile.TileContext(nc) as tc:
        with (
            tc.tile_pool(name="consts", bufs=1) as consts,
            tc.tile_pool(name="loads", bufs=3) as loads,
            tc.tile_pool(name="acts", bufs=3) as acts,
            tc.tile_pool(name="prods", bufs=2) as prods,
            tc.tile_pool(name="outs", bufs=2) as outs,
            tc.tile_pool(name="ps_pre", bufs=3, space="PSUM") as ps_pre,
            tc.tile_pool(name="ps_rep", bufs=2, space="PSUM") as ps_rep,
        ):
            s0 = consts.tile([32, 16], FP32)
            r8 = consts.tile([16, 128], FP32)
            m1a = consts.tile([128, 16], FP32)
            m1b = consts.tile([128, 16], FP32)
            r3 = consts.tile([16, 48], FP32)
            m2 = consts.tile([48, 3], FP32)
            ones1 = consts.tile([1, 3], FP32)
            for t, d in ((s0, c_s0), (r8, c_r8), (m1a, c_m1a), (m1b, c_m1b),
                         (r3, c_r3), (m2, c_m2), (ones1, c_ones1)):
                nc.sync.dma_start(out=t[:], in_=d[:])

            def body():
                for c in range(N_CHUNKS):
                    px = slice(c * F, (c + 1) * F)

                    t0 = loads.tile([32, F], FP32, tag="t0")
                    t1a = loads.tile([128, F], FP32, tag="t1a")
                    t1b = loads.tile([128, F], FP32, tag="t1b")
                    t2 = loads.tile([48, F], FP32, tag="t2")
                    tbias = loads.tile([1, F], FP32, tag="tbias")
                    nc.sync.dma_start(out=t0[:], in_=w[0:32, px])
                    nc.sync.dma_start(out=t1a[:], in_=w[32:160, px])
                    nc.sync.dma_start(out=t1b[:], in_=w[160:288, px])
                    nc.sync.dma_start(out=t2[:], in_=w[288:336, px])
                    nc.sync.dma_start(out=tbias[:], in_=w[336:337, px])

                    # ---- layer 0: h1 = relu(S0.T @ w0) --------------------
                    h1pre = ps_pre.tile([16, F], FP32, tag="pre")
                    nc.tensor.matmul(h1pre[:], s0[:], t0[:], start=True, stop=True)
                    h1 = acts.tile([16, F], FP32, tag="h1")
                    nc.scalar.activation(h1[:], h1pre[:],
                                         mybir.ActivationFunctionType.Relu)

                    # ---- layer 1: h2 = relu(sum_i w1[j,i] * h1[i]) --------
                    h1rep = ps_rep.tile([128, F], FP32, tag="rep")
                    nc.tensor.matmul(h1rep[:], r8[:], h1[:], start=True, stop=True)
                    prodA = prods.tile([128, F], FP32, tag="prodA")
                    prodB = prods.tile([128, F], FP32, tag="prodB")
                    nc.vector.tensor_mul(prodA[:], t1a[:], h1rep[:])
                    nc.vector.tensor_mul(prodB[:], t1b[:], h1rep[:])
                    h2pre = ps_pre.tile([16, F], FP32, tag="pre")
                    nc.tensor.matmul(h2pre[:], m1a[:], prodA[:], start=True, stop=False)
                    nc.tensor.matmul(h2pre[:], m1b[:], prodB[:], start=False, stop=True)
                    h2 = acts.tile([16, F], FP32, tag="h2")
                    nc.scalar.activation(h2[:], h2pre[:],
                                         mybir.ActivationFunctionType.Relu)

                    # ---- layer 2: y = sum_i w2[j,i] * h2[i] + bias --------
                    h2rep = ps_rep.tile([48, F], FP32, tag="rep")
                    nc.tensor.matmul(h2rep[:], r3[:], h2[:], start=True, stop=True)
                    prodC = prods.tile([48, F], FP32, tag="prodC")
                    nc.vector.tensor_mul(prodC[:], t2[:], h2rep[:])
                    y = ps_pre.tile([3, F], FP32, tag="pre")
                    nc.tensor.matmul(y[:], m2[:], prodC[:], start=True, stop=False)
                    nc.tensor.matmul(y[:], ones1[:], tbias[:], start=False,
                                     stop=True)

                    y_sb = outs.tile([3, F], FP32, tag="ysb")
                    nc.scalar.copy(y_sb[:], y[:])
                    nc.sync.dma_start(out=out[:, px], in_=y_sb[:])

            if repeat == 1:
                body()
            else:
                with tc.For_i(0, repeat, 1):
                    body()

    nc.compile()
    return nc


_NC_CACHE: dict[int, object] = {}


def _get_nc(repeat: int = 1):
    if repeat not in _NC_CACHE:
        _NC_CACHE[repeat] = build_nc(repeat)
    return _NC_CACHE[repeat]


def make_in_maps(weight: np.ndarray, coor: np.ndarray) -> list[dict]:
    mats = _const_mats(coor)
    in_maps = []
    for k in range(N_CORES):
        b, hh = k // 2, k % 2
        shard = np.ascontiguousarray(
            weight[b, :, hh * 128:(hh + 1) * 128, :].reshape(N_CH, PIX),
            dtype=np.float32)
        in_maps.append({"w": shard, **mats})
    return in_maps


def assemble_out(results: list[dict]) -> np.ndarray:
    out = np.empty((B, 3, H, W), np.float32)
    for k in range(N_CORES):
        b, hh = k // 2, k % 2
        out[b, :, hh * 128:(hh + 1) * 128, :] = results[k]["out"].reshape(3, 128, W)
    return out


def kernel(input: np.ndarray, weight: np.ndarray, coor: np.ndarray) -> np.ndarray:
    nc = _get_nc(1)
    in_maps = make_in_maps(np.asarray(weight), np.asarray(coor))
    res = run_bass_kernel_spmd(nc, in_maps, core_ids=list(range(N_CORES)))
    return assemble_out(res.results)


# revision 8
# speedup vs baseline: 86.7666x; 1.5392x over previous
"""Trainium2 Bass kernel for the per-pixel locally-connected MLP (dense_mlp).

Reference computation (per batch b, pixel (h,w)):
    x0 = coor (2-vector, shared by all pixels)
    h1 = relu(W0 @ x0)        W0 = weight[b, 0:32].reshape(16, 2)   per pixel
    h2 = relu(W1 @ h1)        W1 = weight[b, 32:288].reshape(16,16) per pixel
    y  = W2 @ h2 + bias       W2 = weight[b, 288:336].reshape(3,16), bias = weight[b,336]
Output: [4, 3, 256, 256] float32.

Sharding: 8 cores, core k handles batch k//2, rows (k%2)*128:(k%2+1)*128
=> per-core weight shard [337, 32768] (channels x pixels), no cross-core comm.

On-chip layout: channels on SBUF partitions, pixels on the free axis.
Per-pixel matvecs = elementwise multiplies (VectorE) + partition-axis
reductions (TensorE matmuls against small 0/1 selection matrices built on the
host; `coor` is folded into the first matmul's stationary matrix).
"""

import sys

for _p in ("/opt/trn_rl_repo", "/root/.axon_site/_ro/trn_rl_repo"):
    if _p not in sys.path:
        sys.path.append(_p)

import numpy as np

import concourse.bass as bass
import concourse.tile as tile
from concourse import bacc, mybir
from concourse.bass_utils import run_bass_kernel_spmd

# ---------------------------------------------------------------- constants
B, H, W = 4, 256, 256
N_CH = 337            # 32 (L0) + 256 (L1) + 48 (L2) + 1 (bias)
N_CORES = 8
PIX = (B * H * W) // N_CORES  # 32768 pixels per core
F = 512               # pixels per compute chunk (one PSUM bank of fp32)
N_CHUNKS = PIX // F

FP32 = mybir.dt.float32


def _const_mats(coor: np.ndarray) -> dict[str, np.ndarray]:
    """Small stationary matrices for the TensorE reductions."""
    cx, cy = float(coor[0]), float(coor[1])
    s0 = np.zeros((32, 16), np.float32)       # h1pre = S0.T @ w[0:32]
    for i in range(16):
        s0[2 * i, i] = cx
        s0[2 * i + 1, i] = cy
    r8 = np.zeros((16, 128), np.float32)      # h1rep[m] = h1[m % 16]
    for m in range(128):
        r8[m % 16, m] = 1.0
    m1a = np.zeros((128, 16), np.float32)     # h2pre[j] += sum_i prodA[16j+i]
    m1b = np.zeros((128, 16), np.float32)
    for k in range(128):
        m1a[k, k // 16] = 1.0
        m1b[k, 8 + k // 16] = 1.0
    r3 = np.zeros((16, 48), np.float32)       # h2rep[m] = h2[m % 16]
    for m in range(48):
        r3[m % 16, m] = 1.0
    m2 = np.zeros((48, 3), np.float32)        # y[j] = sum_i prodC[16j+i]
    for k in range(48):
        m2[k, k // 16] = 1.0
    ones1 = np.ones((1, 3), np.float32)       # y[j] += bias
    return {"s0": s0, "r8": r8, "m1a": m1a, "m1b": m1b, "r3": r3, "m2": m2,
            "ones1": ones1}


def build_nc(repeat: int = 1):
    """Build the per-core Bass program. `repeat` re-runs the whole kernel
    body sequentially (used only for differential HW timing)."""
    nc = bacc.Bacc(None, target_bir_lowering=False)

    w = nc.declare_dram_parameter("w", [N_CH, PIX], FP32, isOutput=False)
    out = nc.declare_dram_parameter("out", [3, PIX], FP32, isOutput=True)
    c_s0 = nc.declare_dram_parameter("s0", [32, 16], FP32, isOutput=False)
    c_r8 = nc.declare_dram_parameter("r8", [16, 128], FP32, isOutput=False)
    c_m1a = nc.declare_dram_parameter("m1a", [128, 16], FP32, isOutput=False)
    c_m1b = nc.declare_dram_parameter("m1b", [128, 16], FP32, isOutput=False)
    c_r3 = nc.declare_dram_parameter("r3", [16, 48], FP32, isOutput=False)
    c_m2 = nc.declare_dram_parameter("m2", [48, 3], FP32, isOutput=False)
    c_ones1 = nc.declare_dram_parameter("ones1", [1, 3], FP32, isOutput=False)

    G = 4                      # chunks per software-pipeline group
    with tile.TileContext(nc) as tc:
        with (
            tc.tile_pool(name="consts", bufs=1) as consts,
            tc.tile_pool(name="loads", bufs=5) as loads,
            tc.tile_pool(name="acts", bufs=4) as acts,
            tc.tile_pool(name="prods", bufs=4) as prods,
            tc.tile_pool(name="outs", bufs=2) as outs,
            tc.tile_pool(name="ps_sm16", bufs=3, space="PSUM") as ps_sm16,
            tc.tile_pool(name="ps_h2p", bufs=2, space="PSUM") as ps_h2p,
            tc.tile_pool(name="ps_rep", bufs=3, space="PSUM") as ps_rep,
        ):
            s0 = consts.tile([32, 16], FP32)
            r8 = consts.tile([16, 128], FP32)
            m1a = consts.tile([128, 16], FP32)
            m1b = consts.tile([128, 16], FP32)
            r3 = consts.tile([16, 48], FP32)
            m2 = consts.tile([48, 3], FP32)
            ones1 = consts.tile([1, 3], FP32)
            for t, d in ((s0, c_s0), (r8, c_r8), (m1a, c_m1a), (m1b, c_m1b),
                         (r3, c_r3), (m2, c_m2), (ones1, c_ones1)):
                nc.sync.dma_start(out=t[:], in_=d[:])

            relu = mybir.ActivationFunctionType.Relu

            def body():
                # Stage-major emission over groups of G chunks: each engine
                # gets bursts of independent per-chunk work, so the in-order
                # PE/ACT/DVE queues pipeline across chunks instead of
                # waiting on same-chunk cross-engine results.
                for g in range(N_CHUNKS // G):
                    cs = [g * G + i for i in range(G)]
                    pxs = [slice(c * F, (c + 1) * F) for c in cs]

                    t0, t1a, t1b, t2, tb = {}, {}, {}, {}, {}
                    for i, c in enumerate(cs):
                        t0[c] = loads.tile([32, F], FP32, tag="t0", name="t0")
                        t1a[c] = loads.tile([128, F], FP32, tag="t1a", name="t1a")
                        t1b[c] = loads.tile([128, F], FP32, tag="t1b", name="t1b")
                        t2[c] = loads.tile([48, F], FP32, tag="t2", name="t2")
                        tb[c] = loads.tile([1, F], FP32, tag="tbias", name="tbias")
                        nc.sync.dma_start(out=t0[c][:], in_=w[0:32, pxs[i]])
                        nc.sync.dma_start(out=t1a[c][:], in_=w[32:160, pxs[i]])
                        nc.sync.dma_start(out=t1b[c][:], in_=w[160:288, pxs[i]])
                        nc.sync.dma_start(out=t2[c][:], in_=w[288:336, pxs[i]])
                        nc.sync.dma_start(out=tb[c][:], in_=w[336:337, pxs[i]])

                    h1pre = {}
                    for c in cs:
                        h1pre[c] = ps_sm16.tile([16, F], FP32, tag="sm16", name="h1pre")
                        nc.tensor.matmul(h1pre[c][:], s0[:], t0[c][:],
                                         start=True, stop=True)
                    h1 = {}
                    for c in cs:
                        h1[c] = acts.tile([16, F], FP32, tag="h1", name="h1")
                        nc.scalar.activation(h1[c][:], h1pre[c][:], relu)
                    h1rep = {}
                    for c in cs:
                        h1rep[c] = ps_rep.tile([128, F], FP32, tag="rep", name="h1rep")
                        nc.tensor.matmul(h1rep[c][:], r8[:], h1[c][:],
                                         start=True, stop=True)
                    prodA, prodB = {}, {}
                    for c in cs:
                        prodA[c] = prods.tile([128, F], FP32, tag="prodA", name="prodA")
                        prodB[c] = prods.tile([128, F], FP32, tag="prodB", name="prodB")
                        nc.vector.tensor_mul(prodA[c][:], t1a[c][:], h1rep[c][:])
                        nc.vector.tensor_mul(prodB[c][:], t1b[c][:], h1rep[c][:])
                    h2pre = {}
                    for c in cs:
                        h2pre[c] = ps_h2p.tile([16, F], FP32, tag="h2p", name="h2pre")
                        nc.tensor.matmul(h2pre[c][:], m1a[:], prodA[c][:],
                                         start=True, stop=False)
                        nc.tensor.matmul(h2pre[c][:], m1b[:], prodB[c][:],
                                         start=False, stop=True)
                    h2 = {}
                    for c in cs:
                        h2[c] = acts.tile([16, F], FP32, tag="h2", name="h2")
                        nc.scalar.activation(h2[c][:], h2pre[c][:], relu)
                    h2rep = {}
                    for c in cs:
                        h2rep[c] = ps_rep.tile([48, F], FP32, tag="rep", name="h2rep")
                        nc.tensor.matmul(h2rep[c][:], r3[:], h2[c][:],
                                         start=True, stop=True)
                    prodC = {}
                    for c in cs:
                        prodC[c] = prods.tile([48, F], FP32, tag="prodC", name="prodC")
                        nc.vector.tensor_mul(prodC[c][:], t2[c][:], h2rep[c][:])
                    y = {}
                    for c in cs:
                        y[c] = ps_sm16.tile([16, F], FP32, tag="sm16", name="y")
                        nc.tensor.matmul(y[c][0:3, :], m2[:], prodC[c][:],
                                         start=True, stop=False)
                        nc.tensor.matmul(y[c][0:3, :], ones1[:], tb[c][:],
                                         start=False, stop=True)
                    y_sb = outs.tile([3, G * F], FP32, tag="ysb", name="ysb")
                    for i, c in enumerate(cs):
                        nc.scalar.copy(y_sb[:, i * F:(i + 1) * F], y[c][0:3, :])
                    nc.sync.dma_start(out=out[:, cs[0] * F:(cs[-1] + 1) * F],
                                      in_=y_sb[:])

            if repeat == 1:
                body()
            else:
                with tc.For_i(0, repeat, 1):
                    body()

    nc.compile()
    return nc


_NC_CACHE: dict[int, object] = {}


def _get_nc(repeat: int = 1):
    if repeat not in _NC_CACHE:
        _NC_CACHE[repeat] = build_nc(repeat)
    return _NC_CACHE[repeat]


def make_in_maps(weight: np.ndarray, coor: np.ndarray) -> list[dict]:
    mats = _const_mats(coor)
    in_maps = []
    for k in range(N_CORES):
        b, hh = k // 2, k % 2
        shard = np.ascontiguousarray(
            weight[b, :, hh * 128:(hh + 1) * 128, :].reshape(N_CH, PIX),
            dtype=np.float32)
        in_maps.append({"w": shard, **mats})
    return in_maps


def assemble_out(results: list[dict]) -> np.ndarray:
    out = np.empty((B, 3, H, W), np.float32)
    for k in range(N_CORES):
        b, hh = k // 2, k % 2
        out[b, :, hh * 128:(hh + 1) * 128, :] = results[k]["out"].reshape(3, 128, W)
    return out


def kernel(input: np.ndarray, weight: np.ndarray, coor: np.ndarray) -> np.ndarray:
    nc = _get_nc(1)
    in_maps = make_in_maps(np.asarray(weight), np.asarray(coor))
    res = run_bass_kernel_spmd(nc, in_maps, core_ids=list(range(N_CORES)))
    return assemble_out(res.results)


# revision 11
# speedup vs baseline: 183.7198x; 2.1174x over previous
"""Trainium2 Bass kernel for the per-pixel locally-connected MLP (dense_mlp).

Reference computation (per batch b, pixel (h,w)):
    x0 = coor (2-vector, shared by all pixels)
    h1 = relu(W0 @ x0)        W0 = weight[b, 0:32].reshape(16, 2)   per pixel
    h2 = relu(W1 @ h1)        W1 = weight[b, 32:288].reshape(16,16) per pixel
    y  = W2 @ h2 + bias       W2 = weight[b, 288:336].reshape(3,16), bias = weight[b,336]
Output: [4, 3, 256, 256] float32.

Sharding: 8 cores, core k handles batch k//2, rows (k%2)*128:(k%2+1)*128
=> per-core weight shard [337, 32768] (channels x pixels), no cross-core comm.

On-chip layout: channels on SBUF partitions, pixels on the free axis.
Per-pixel matvecs = elementwise multiplies (VectorE) + partition-axis
reductions (TensorE matmuls against small 0/1 selection matrices built on the
host; `coor` is folded into the first matmul's stationary matrix).
"""

import sys

for _p in ("/opt/trn_rl_repo", "/root/.axon_site/_ro/trn_rl_repo"):
    if _p not in sys.path:
        sys.path.append(_p)

import numpy as np

import concourse.bass as bass
import concourse.tile as tile
from concourse import bacc, mybir
from concourse.bass_utils import run_bass_kernel_spmd

# ---------------------------------------------------------------- constants
B, H, W = 4, 256, 256
N_CH = 337            # 32 (L0) + 256 (L1) + 48 (L2) + 1 (bias)
N_CORES = 8
PIX = (B * H * W) // N_CORES  # 32768 pixels per core
F = 512               # pixels per compute chunk (one PSUM bank of fp32)
N_CHUNKS = PIX // F

FP32 = mybir.dt.float32
FP32R = mybir.dt.float32r


def _f(ap):
    return ap.bitcast(FP32)


def _const_mats(coor: np.ndarray) -> dict[str, np.ndarray]:
    """Small stationary matrices for the TensorE reductions."""
    cx, cy = float(coor[0]), float(coor[1])
    s0 = np.zeros((32, 16), np.float32)       # h1pre = S0.T @ w[0:32]
    for i in range(16):
        s0[2 * i, i] = cx
        s0[2 * i + 1, i] = cy
    r8 = np.zeros((16, 128), np.float32)      # h1rep[m] = h1[m % 16]
    for m in range(128):
        r8[m % 16, m] = 1.0
    m1a = np.zeros((128, 16), np.float32)     # h2pre[j] += sum_i prodA[16j+i]
    m1b = np.zeros((128, 16), np.float32)
    for k in range(128):
        m1a[k, k // 16] = 1.0
        m1b[k, 8 + k // 16] = 1.0
    r3 = np.zeros((16, 48), np.float32)       # h2rep[m] = h2[m % 16]
    for m in range(48):
        r3[m % 16, m] = 1.0
    m2 = np.zeros((48, 3), np.float32)        # y[j] = sum_i prodC[16j+i]
    for k in range(48):
        m2[k, k // 16] = 1.0
    ones1 = np.ones((1, 3), np.float32)       # y[j] += bias
    return {"s0": s0, "r8": r8, "m1a": m1a, "m1b": m1b, "r3": r3, "m2": m2,
            "ones1": ones1}


def build_nc(repeat: int = 1):
    """Build the per-core Bass program. `repeat` re-runs the whole kernel
    body sequentially (used only for differential HW timing)."""
    nc = bacc.Bacc(None, target_bir_lowering=False)

    w = nc.declare_dram_parameter("w", [N_CH, PIX], FP32R, isOutput=False)
    out = nc.declare_dram_parameter("out", [3, PIX], FP32, isOutput=True)
    c_s0 = nc.declare_dram_parameter("s0", [32, 16], FP32R, isOutput=False)
    c_r8 = nc.declare_dram_parameter("r8", [16, 128], FP32R, isOutput=False)
    c_m1a = nc.declare_dram_parameter("m1a", [128, 16], FP32R, isOutput=False)
    c_m1b = nc.declare_dram_parameter("m1b", [128, 16], FP32R, isOutput=False)
    c_r3 = nc.declare_dram_parameter("r3", [16, 48], FP32R, isOutput=False)
    c_m2 = nc.declare_dram_parameter("m2", [48, 3], FP32R, isOutput=False)
    c_ones1 = nc.declare_dram_parameter("ones1", [1, 3], FP32R, isOutput=False)

    G = 4                      # chunks per software-pipeline group
    with tile.TileContext(nc) as tc:
        with (
            tc.tile_pool(name="consts", bufs=1) as consts,
            tc.tile_pool(name="loads", bufs=5) as loads,
            tc.tile_pool(name="acts", bufs=4) as acts,
            tc.tile_pool(name="prods", bufs=4) as prods,
            tc.tile_pool(name="outs", bufs=2) as outs,
            tc.tile_pool(name="ps_sm16", bufs=3, space="PSUM") as ps_sm16,
            tc.tile_pool(name="ps_h2p", bufs=2, space="PSUM") as ps_h2p,
            tc.tile_pool(name="ps_rep", bufs=3, space="PSUM") as ps_rep,
        ):
            s0 = consts.tile([32, 16], FP32R)
            r8 = consts.tile([16, 128], FP32R)
            m1a = consts.tile([128, 16], FP32R)
            m1b = consts.tile([128, 16], FP32R)
            r3 = consts.tile([16, 48], FP32R)
            m2 = consts.tile([48, 3], FP32R)
            ones1 = consts.tile([1, 3], FP32R)
            for t, d in ((s0, c_s0), (r8, c_r8), (m1a, c_m1a), (m1b, c_m1b),
                         (r3, c_r3), (m2, c_m2), (ones1, c_ones1)):
                nc.sync.dma_start(out=t[:], in_=d[:])

            relu = mybir.ActivationFunctionType.Relu

            def body():
                # Stage-major emission over groups of G chunks: each engine
                # gets bursts of independent per-chunk work, so the in-order
                # PE/ACT/DVE queues pipeline across chunks instead of
                # waiting on same-chunk cross-engine results.
                for g in range(N_CHUNKS // G):
                    cs = [g * G + i for i in range(G)]
                    pxs = [slice(c * F, (c + 1) * F) for c in cs]

                    t0, t1a, t1b, t2, tb = {}, {}, {}, {}, {}
                    for i, c in enumerate(cs):
                        t0[c] = loads.tile([32, F], FP32R, tag="t0", name="t0")
                        t1a[c] = loads.tile([128, F], FP32R, tag="t1a", name="t1a")
                        t1b[c] = loads.tile([128, F], FP32R, tag="t1b", name="t1b")
                        t2[c] = loads.tile([48, F], FP32R, tag="t2", name="t2")
                        tb[c] = loads.tile([1, F], FP32R, tag="tbias", name="tbias")
                        nc.sync.dma_start(out=t0[c][:], in_=w[0:32, pxs[i]])
                        nc.sync.dma_start(out=t1a[c][:], in_=w[32:160, pxs[i]])
                        nc.sync.dma_start(out=t1b[c][:], in_=w[160:288, pxs[i]])
                        nc.sync.dma_start(out=t2[c][:], in_=w[288:336, pxs[i]])
                        nc.sync.dma_start(out=tb[c][:], in_=w[336:337, pxs[i]])

                    h1pre = {}
                    for c in cs:
                        h1pre[c] = ps_sm16.tile([16, F], FP32, tag="sm16", name="h1pre")
                        nc.tensor.matmul(h1pre[c][:], s0[:], t0[c][:],
                                         start=True, stop=True)
                    h1 = {}
                    for c in cs:
                        h1[c] = acts.tile([16, F], FP32R, tag="h1", name="h1")
                        nc.scalar.activation(h1[c][:], h1pre[c][:], relu)
                    h1rep = {}
                    for c in cs:
                        h1rep[c] = ps_rep.tile([128, F], FP32, tag="rep", name="h1rep")
                        nc.tensor.matmul(h1rep[c][:], r8[:], h1[c][:],
                                         start=True, stop=True)
                    prodA, prodB = {}, {}
                    for c in cs:
                        prodA[c] = prods.tile([128, F], FP32R, tag="prodA", name="prodA")
                        prodB[c] = prods.tile([128, F], FP32R, tag="prodB", name="prodB")
                        nc.vector.tensor_mul(prodA[c][:], _f(t1a[c][:]), h1rep[c][:])
                        nc.vector.tensor_mul(prodB[c][:], _f(t1b[c][:]), h1rep[c][:])
                    h2pre = {}
                    for c in cs:
                        h2pre[c] = ps_h2p.tile([16, F], FP32, tag="h2p", name="h2pre")
                        nc.tensor.matmul(h2pre[c][:], m1a[:], prodA[c][:],
                                         start=True, stop=False)
                        nc.tensor.matmul(h2pre[c][:], m1b[:], prodB[c][:],
                                         start=False, stop=True)
                    h2 = {}
                    for c in cs:
                        h2[c] = acts.tile([16, F], FP32R, tag="h2", name="h2")
                        nc.scalar.activation(h2[c][:], h2pre[c][:], relu)
                    h2rep = {}
                    for c in cs:
                        h2rep[c] = ps_rep.tile([48, F], FP32, tag="rep", name="h2rep")
                        nc.tensor.matmul(h2rep[c][:], r3[:], h2[c][:],
                                         start=True, stop=True)
                    prodC = {}
                    for c in cs:
                        prodC[c] = prods.tile([48, F], FP32R, tag="prodC", name="prodC")
                        nc.vector.tensor_mul(prodC[c][:], _f(t2[c][:]), h2rep[c][:])
                    y = {}
                    for c in cs:
                        y[c] = ps_sm16.tile([16, F], FP32, tag="sm16", name="y")
                        nc.tensor.matmul(y[c][0:3, :], m2[:], prodC[c][:],
                                         start=True, stop=False)
                        nc.tensor.matmul(y[c][0:3, :], ones1[:], tb[c][:],
                                         start=False, stop=True)
                    y_sb = outs.tile([3, G * F], FP32, tag="ysb", name="ysb")
                    for i, c in enumerate(cs):
                        nc.scalar.copy(y_sb[:, i * F:(i + 1) * F], y[c][0:3, :])
                    nc.sync.dma_start(out=out[:, cs[0] * F:(cs[-1] + 1) * F],
                                      in_=y_sb[:])

            if repeat == 1:
                body()
            else:
                with tc.For_i(0, repeat, 1):
                    body()

    nc.compile()
    return nc


_NC_CACHE: dict[int, object] = {}


def _get_nc(repeat: int = 1):
    if repeat not in _NC_CACHE:
        _NC_CACHE[repeat] = build_nc(repeat)
    return _NC_CACHE[repeat]


def make_in_maps(weight: np.ndarray, coor: np.ndarray) -> list[dict]:
    mats = _const_mats(coor)
    in_maps = []
    for k in range(N_CORES):
        b, hh = k // 2, k % 2
        shard = np.ascontiguousarray(
            weight[b, :, hh * 128:(hh + 1) * 128, :].reshape(N_CH, PIX),
            dtype=np.float32)
        in_maps.append({"w": shard, **mats})
    return in_maps


def assemble_out(results: list[dict]) -> np.ndarray:
    out = np.empty((B, 3, H, W), np.float32)
    for k in range(N_CORES):
        b, hh = k // 2, k % 2
        out[b, :, hh * 128:(hh + 1) * 128, :] = results[k]["out"].reshape(3, 128, W)
    return out


def kernel(input: np.ndarray, weight: np.ndarray, coor: np.ndarray) -> np.ndarray:
    nc = _get_nc(1)
    in_maps = make_in_maps(np.asarray(weight), np.asarray(coor))
    res = run_bass_kernel_spmd(nc, in_maps, core_ids=list(range(N_CORES)))
    return assemble_out(res.results)


# revision 13
# speedup vs baseline: 254.9849x; 1.3879x over previous
"""Trainium2 Bass kernel for the per-pixel locally-connected MLP (dense_mlp).

Reference computation (per batch b, pixel (h,w)):
    x0 = coor (2-vector, shared by all pixels)
    h1 = relu(W0 @ x0)        W0 = weight[b, 0:32].reshape(16, 2)   per pixel
    h2 = relu(W1 @ h1)        W1 = weight[b, 32:288].reshape(16,16) per pixel
    y  = W2 @ h2 + bias       W2 = weight[b, 288:336].reshape(3,16), bias = weight[b,336]
Output: [4, 3, 256, 256] float32.

Sharding: 8 cores, core k handles batch k//2, rows (k%2)*128:(k%2+1)*128
=> per-core weight shard [337, 32768] (channels x pixels), no cross-core comm.

On-chip layout: channels on SBUF partitions, pixels on the free axis.
Per-pixel matvecs = elementwise multiplies (VectorE) + partition-axis
reductions (TensorE matmuls against small 0/1 selection matrices built on the
host; `coor` is folded into the first matmul's stationary matrix).
"""

import sys

for _p in ("/opt/trn_rl_repo", "/root/.axon_site/_ro/trn_rl_repo"):
    if _p not in sys.path:
        sys.path.append(_p)

import numpy as np

import concourse.bass as bass
import concourse.tile as tile
from concourse import bacc, mybir
from concourse.bass_utils import run_bass_kernel_spmd

# ---------------------------------------------------------------- constants
B, H, W = 4, 256, 256
N_CH = 337            # 32 (L0) + 256 (L1) + 48 (L2) + 1 (bias)
N_CORES = 8
PIX = (B * H * W) // N_CORES  # 32768 pixels per core
F = 512               # pixels per compute chunk (one PSUM bank of fp32)
N_CHUNKS = PIX // F

FP32 = mybir.dt.float32
FP32R = mybir.dt.float32r


def _f(ap):
    return ap.bitcast(FP32)


def _const_mats(coor: np.ndarray) -> dict[str, np.ndarray]:
    """Small stationary matrices for the TensorE reductions."""
    cx, cy = float(coor[0]), float(coor[1])
    s0 = np.zeros((32, 16), np.float32)       # h1pre = S0.T @ w[0:32]
    for i in range(16):
        s0[2 * i, i] = cx
        s0[2 * i + 1, i] = cy
    r8 = np.zeros((16, 128), np.float32)      # h1rep[m] = h1[m % 16]
    for m in range(128):
        r8[m % 16, m] = 1.0
    m1a = np.zeros((128, 16), np.float32)     # h2pre[j] += sum_i prodA[16j+i]
    m1b = np.zeros((128, 16), np.float32)
    for k in range(128):
        m1a[k, k // 16] = 1.0
        m1b[k, 8 + k // 16] = 1.0
    r3 = np.zeros((16, 48), np.float32)       # h2rep[m] = h2[m % 16]
    for m in range(48):
        r3[m % 16, m] = 1.0
    m2b = np.zeros((49, 3), np.float32)       # y[j] = sum_i prodC[16j+i] + bias
    for k in range(48):
        m2b[k, k // 16] = 1.0
    m2b[48, :] = 1.0                          # row 48 of prodC holds the bias
    return {"s0": s0, "r8": r8, "m1a": m1a, "m1b": m1b, "r3": r3, "m2b": m2b}


def build_nc(repeat: int = 1):
    """Build the per-core Bass program. `repeat` re-runs the whole kernel
    body sequentially (used only for differential HW timing)."""
    nc = bacc.Bacc(None, target_bir_lowering=False)

    w = nc.declare_dram_parameter("w", [N_CH, PIX], FP32R, isOutput=False)
    out = nc.declare_dram_parameter("out", [3, PIX], FP32, isOutput=True)
    c_s0 = nc.declare_dram_parameter("s0", [32, 16], FP32R, isOutput=False)
    c_r8 = nc.declare_dram_parameter("r8", [16, 128], FP32R, isOutput=False)
    c_m1a = nc.declare_dram_parameter("m1a", [128, 16], FP32R, isOutput=False)
    c_m1b = nc.declare_dram_parameter("m1b", [128, 16], FP32R, isOutput=False)
    c_r3 = nc.declare_dram_parameter("r3", [16, 48], FP32R, isOutput=False)
    c_m2b = nc.declare_dram_parameter("m2b", [49, 3], FP32R, isOutput=False)

    G = 4                      # chunks per software-pipeline group
    with tile.TileContext(nc) as tc:
        with (
            tc.tile_pool(name="consts", bufs=1) as consts,
            tc.tile_pool(name="loads", bufs=2) as loads,
            tc.tile_pool(name="acts", bufs=4) as acts,
            tc.tile_pool(name="prods", bufs=3) as prods,
            tc.tile_pool(name="outs", bufs=2) as outs,
            tc.tile_pool(name="ps_sm16", bufs=3, space="PSUM") as ps_sm16,
            tc.tile_pool(name="ps_h2p", bufs=2, space="PSUM") as ps_h2p,
            tc.tile_pool(name="ps_rep", bufs=3, space="PSUM") as ps_rep,
        ):
            s0 = consts.tile([32, 16], FP32R)
            r8 = consts.tile([16, 128], FP32R)
            m1a = consts.tile([128, 16], FP32R)
            m1b = consts.tile([128, 16], FP32R)
            r3 = consts.tile([16, 48], FP32R)
            m2b = consts.tile([49, 3], FP32R)
            for t, d in ((s0, c_s0), (r8, c_r8), (m1a, c_m1a), (m1b, c_m1b),
                         (r3, c_r3), (m2b, c_m2b)):
                nc.sync.dma_start(out=t[:], in_=d[:])

            relu = mybir.ActivationFunctionType.Relu

            def body():
                # Per 2048-px macro-tile: 3 big HWDGE loads (sync) + bias and
                # output DMAs on gpsimd (SWDGE), then 4 sub-chunks of 512 px
                # emitted stage-major so the in-order engine queues pipeline
                # across sub-chunks.
                FM = G * F     # macro-tile pixel count
                for g in range(N_CHUNKS // G):
                    mp = slice(g * FM, (g + 1) * FM)
                    sls = [slice(i * F, (i + 1) * F) for i in range(G)]

                    t0m = loads.tile([32, FM], FP32R, tag="t0", name="t0m")
                    t1m = loads.tile([128, 2, FM], FP32R, tag="t1", name="t1m")
                    t2m = loads.tile([48, FM], FP32R, tag="t2", name="t2m")
                    # prodC macro-tile: rows 0:48 = w2*h2rep (DVE, per chunk),
                    # row 48 = bias channel (DMA); one matmul does L2 + bias.
                    pcm = prods.tile([49, FM], FP32R, tag="pcm", name="pcm", bufs=2)
                    nc.sync.dma_start(out=t0m[:], in_=w[0:32, mp])
                    nc.sync.dma_start(
                        out=t1m[:],
                        in_=w[32:288, mp].rearrange("(b p) x -> p b x", b=2))
                    nc.sync.dma_start(out=t2m[:], in_=w[288:336, mp])
                    nc.gpsimd.dma_start(out=pcm[48:49, :], in_=w[336:337, mp])

                    h1pre = {}
                    for i in range(G):
                        h1pre[i] = ps_sm16.tile([16, F], FP32, tag="sm16", name="h1pre")
                        nc.tensor.matmul(h1pre[i][:], s0[:], t0m[:, sls[i]],
                                         start=True, stop=True)
                    h1 = {}
                    for i in range(G):
                        h1[i] = acts.tile([16, F], FP32R, tag="h1", name="h1")
                        nc.scalar.activation(h1[i][:], h1pre[i][:], relu)
                    h1rep = {}
                    for i in range(G):
                        h1rep[i] = ps_rep.tile([128, F], FP32, tag="rep", name="h1rep")
                        nc.tensor.matmul(h1rep[i][:], r8[:], h1[i][:],
                                         start=True, stop=True)
                    prodAB = {}
                    for i in range(G):
                        prodAB[i] = prods.tile([128, 2, F], FP32R, tag="prodAB",
                                               name="prodAB")
                        rep2 = bass.AP(tensor=h1rep[i].tensor,
                                       offset=h1rep[i][:].offset,
                                       ap=[h1rep[i][:].ap[0], [0, 2],
                                           h1rep[i][:].ap[1]])
                        nc.vector.tensor_mul(prodAB[i][:],
                                             _f(t1m[:, :, sls[i]]), rep2)
                    h2pre = {}
                    for i in range(G):
                        h2pre[i] = ps_h2p.tile([16, F], FP32, tag="h2p", name="h2pre")
                        nc.tensor.matmul(h2pre[i][:], m1a[:], prodAB[i][:, 0, :],
                                         start=True, stop=False)
                        nc.tensor.matmul(h2pre[i][:], m1b[:], prodAB[i][:, 1, :],
                                         start=False, stop=True)
                    h2 = {}
                    for i in range(G):
                        h2[i] = acts.tile([16, F], FP32R, tag="h2", name="h2")
                        nc.scalar.activation(h2[i][:], h2pre[i][:], relu)
                    h2rep = {}
                    for i in range(G):
                        h2rep[i] = ps_rep.tile([48, F], FP32, tag="rep", name="h2rep")
                        nc.tensor.matmul(h2rep[i][:], r3[:], h2[i][:],
                                         start=True, stop=True)
                    for i in range(G):
                        nc.vector.tensor_mul(pcm[0:48, sls[i]],
                                             _f(t2m[:, sls[i]]), h2rep[i][:])
                    y = {}
                    for i in range(G):
                        y[i] = ps_sm16.tile([16, F], FP32, tag="sm16", name="y")
                        nc.tensor.matmul(y[i][0:3, :], m2b[:], pcm[:, sls[i]],
                                         start=True, stop=True)
                    y_sb = outs.tile([3, FM], FP32, tag="ysb", name="ysb")
                    for i in range(G):
                        nc.scalar.copy(y_sb[:, sls[i]], y[i][0:3, :])
                    nc.gpsimd.dma_start(out=out[:, mp], in_=y_sb[:])

            if repeat == 1:
                body()
            else:
                with tc.For_i(0, repeat, 1):
                    body()

    nc.compile()
    return nc


_NC_CACHE: dict[int, object] = {}


def _get_nc(repeat: int = 1):
    if repeat not in _NC_CACHE:
        _NC_CACHE[repeat] = build_nc(repeat)
    return _NC_CACHE[repeat]


def make_in_maps(weight: np.ndarray, coor: np.ndarray) -> list[dict]:
    mats = _const_mats(coor)
    in_maps = []
    for k in range(N_CORES):
        b, hh = k // 2, k % 2
        shard = np.ascontiguousarray(
            weight[b, :, hh * 128:(hh + 1) * 128, :].reshape(N_CH, PIX),
            dtype=np.float32)
        in_maps.append({"w": shard, **mats})
    return in_maps


def assemble_out(results: list[dict]) -> np.ndarray:
    out = np.empty((B, 3, H, W), np.float32)
    for k in range(N_CORES):
        b, hh = k // 2, k % 2
        out[b, :, hh * 128:(hh + 1) * 128, :] = results[k]["out"].reshape(3, 128, W)
    return out


def kernel(input: np.ndarray, weight: np.ndarray, coor: np.ndarray) -> np.ndarray:
    nc = _get_nc(1)
    in_maps = make_in_maps(np.asarray(weight), np.asarray(coor))
    res = run_bass_kernel_spmd(nc, in_maps, core_ids=list(range(N_CORES)))
    return assemble_out(res.results)


# revision 14
# speedup vs baseline: 314.3567x; 1.2328x over previous
"""Trainium2 Bass kernel for the per-pixel locally-connected MLP (dense_mlp).

Reference computation (per batch b, pixel (h,w)):
    x0 = coor (2-vector, shared by all pixels)
    h1 = relu(W0 @ x0)        W0 = weight[b, 0:32].reshape(16, 2)   per pixel
    h2 = relu(W1 @ h1)        W1 = weight[b, 32:288].reshape(16,16) per pixel
    y  = W2 @ h2 + bias       W2 = weight[b, 288:336].reshape(3,16), bias = weight[b,336]
Output: [4, 3, 256, 256] float32.

Sharding: 8 cores, core k handles batch k//2, rows (k%2)*128:(k%2+1)*128
=> per-core weight shard [337, 32768] (channels x pixels), no cross-core comm.

On-chip layout: channels on SBUF partitions, pixels on the free axis.
Per-pixel matvecs = elementwise multiplies (VectorE) + partition-axis
reductions (TensorE matmuls against small 0/1 selection matrices built on the
host; `coor` is folded into the first matmul's stationary matrix).
"""

import sys

for _p in ("/opt/trn_rl_repo", "/root/.axon_site/_ro/trn_rl_repo"):
    if _p not in sys.path:
        sys.path.append(_p)

import numpy as np

import concourse.bass as bass
import concourse.tile as tile
from concourse import bacc, mybir
from concourse.bass_utils import run_bass_kernel_spmd

# ---------------------------------------------------------------- constants
B, H, W = 4, 256, 256
N_CH = 337            # 32 (L0) + 256 (L1) + 48 (L2) + 1 (bias)
N_CORES = 8
PIX = (B * H * W) // N_CORES  # 32768 pixels per core
F = 512               # pixels per compute chunk (one PSUM bank of fp32)
N_CHUNKS = PIX // F

FP32 = mybir.dt.float32
FP32R = mybir.dt.float32r


def _f(ap):
    return ap.bitcast(FP32)


def _const_mats(coor: np.ndarray) -> dict[str, np.ndarray]:
    """Small stationary matrices for the TensorE reductions."""
    cx, cy = float(coor[0]), float(coor[1])
    s0 = np.zeros((32, 16), np.float32)       # h1pre = S0.T @ w[0:32]
    for i in range(16):
        s0[2 * i, i] = cx
        s0[2 * i + 1, i] = cy
    r8 = np.zeros((16, 128), np.float32)      # h1rep[m] = h1[m % 16]
    for m in range(128):
        r8[m % 16, m] = 1.0
    m1a = np.zeros((128, 16), np.float32)     # h2pre[j] += sum_i prodA[16j+i]
    m1b = np.zeros((128, 16), np.float32)
    for k in range(128):
        m1a[k, k // 16] = 1.0
        m1b[k, 8 + k // 16] = 1.0
    r3 = np.zeros((16, 48), np.float32)       # h2rep[m] = h2[m % 16]
    for m in range(48):
        r3[m % 16, m] = 1.0
    m2b = np.zeros((49, 3), np.float32)       # y[j] = sum_i prodC[16j+i] + bias
    for k in range(48):
        m2b[k, k // 16] = 1.0
    m2b[48, :] = 1.0                          # row 48 of prodC holds the bias
    return {"s0": s0, "r8": r8, "m1a": m1a, "m1b": m1b, "r3": r3, "m2b": m2b}


def build_nc(repeat: int = 1):
    """Build the per-core Bass program. `repeat` re-runs the whole kernel
    body sequentially (used only for differential HW timing)."""
    nc = bacc.Bacc(None, target_bir_lowering=False)

    w = nc.declare_dram_parameter("w", [N_CH, PIX], FP32R, isOutput=False)
    out = nc.declare_dram_parameter("out", [3, PIX], FP32, isOutput=True)
    c_s0 = nc.declare_dram_parameter("s0", [32, 16], FP32R, isOutput=False)
    c_r8 = nc.declare_dram_parameter("r8", [16, 128], FP32R, isOutput=False)
    c_m1a = nc.declare_dram_parameter("m1a", [128, 16], FP32R, isOutput=False)
    c_m1b = nc.declare_dram_parameter("m1b", [128, 16], FP32R, isOutput=False)
    c_r3 = nc.declare_dram_parameter("r3", [16, 48], FP32R, isOutput=False)
    c_m2b = nc.declare_dram_parameter("m2b", [49, 3], FP32R, isOutput=False)

    G = 8                      # chunks per software-pipeline group
    with tile.TileContext(nc) as tc:
        with (
            tc.tile_pool(name="consts", bufs=1) as consts,
            tc.tile_pool(name="loads", bufs=2) as loads,
            tc.tile_pool(name="acts", bufs=3) as acts,
            tc.tile_pool(name="prods", bufs=3) as prods,
            tc.tile_pool(name="outs", bufs=2) as outs,
            tc.tile_pool(name="ps_sm16", bufs=3, space="PSUM") as ps_sm16,
            tc.tile_pool(name="ps_h2p", bufs=2, space="PSUM") as ps_h2p,
            tc.tile_pool(name="ps_rep", bufs=3, space="PSUM") as ps_rep,
        ):
            s0 = consts.tile([32, 16], FP32R)
            r8 = consts.tile([16, 128], FP32R)
            m1a = consts.tile([128, 16], FP32R)
            m1b = consts.tile([128, 16], FP32R)
            r3 = consts.tile([16, 48], FP32R)
            m2b = consts.tile([49, 3], FP32R)
            for t, d in ((s0, c_s0), (r8, c_r8), (m1a, c_m1a), (m1b, c_m1b),
                         (r3, c_r3), (m2b, c_m2b)):
                nc.sync.dma_start(out=t[:], in_=d[:])

            relu = mybir.ActivationFunctionType.Relu

            def body():
                # Per 2048-px macro-tile: 3 big HWDGE loads (sync) + bias and
                # output DMAs on gpsimd (SWDGE), then 4 sub-chunks of 512 px
                # emitted stage-major so the in-order engine queues pipeline
                # across sub-chunks.
                FM = G * F     # macro-tile pixel count
                for g in range(N_CHUNKS // G):
                    mp = slice(g * FM, (g + 1) * FM)
                    sls = [slice(i * F, (i + 1) * F) for i in range(G)]

                    t0m = loads.tile([32, FM], FP32R, tag="t0", name="t0m")
                    t1m = loads.tile([128, 2, FM], FP32R, tag="t1", name="t1m")
                    t2m = loads.tile([48, FM], FP32R, tag="t2", name="t2m")
                    # prodC macro-tile: rows 0:48 = w2*h2rep (DVE, per chunk),
                    # row 48 = bias channel (DMA); one matmul does L2 + bias.
                    pcm = prods.tile([49, FM], FP32R, tag="pcm", name="pcm", bufs=2)
                    nc.sync.dma_start(out=t0m[:], in_=w[0:32, mp])
                    nc.sync.dma_start(
                        out=t1m[:],
                        in_=w[32:288, mp].rearrange("(b p) x -> p b x", b=2))
                    nc.sync.dma_start(out=t2m[:], in_=w[288:336, mp])
                    nc.gpsimd.dma_start(out=pcm[48:49, :], in_=w[336:337, mp])

                    h1pre = {}
                    for i in range(G):
                        h1pre[i] = ps_sm16.tile([16, F], FP32, tag="sm16", name="h1pre")
                        nc.tensor.matmul(h1pre[i][:], s0[:], t0m[:, sls[i]],
                                         start=True, stop=True)
                    h1 = {}
                    for i in range(G):
                        h1[i] = acts.tile([16, F], FP32R, tag="h1", name="h1")
                        nc.scalar.activation(h1[i][:], h1pre[i][:], relu)
                    h1rep = {}
                    for i in range(G):
                        h1rep[i] = ps_rep.tile([128, F], FP32, tag="rep", name="h1rep")
                        nc.tensor.matmul(h1rep[i][:], r8[:], h1[i][:],
                                         start=True, stop=True)
                    prodAB = {}
                    for i in range(G):
                        prodAB[i] = prods.tile([128, 2, F], FP32R, tag="prodAB",
                                               name="prodAB", bufs=2)
                        rep2 = bass.AP(tensor=h1rep[i].tensor,
                                       offset=h1rep[i][:].offset,
                                       ap=[h1rep[i][:].ap[0], [0, 2],
                                           h1rep[i][:].ap[1]])
                        nc.vector.tensor_mul(prodAB[i][:],
                                             _f(t1m[:, :, sls[i]]), rep2)
                    h2pre = {}
                    for i in range(G):
                        h2pre[i] = ps_h2p.tile([16, F], FP32, tag="h2p", name="h2pre")
                        nc.tensor.matmul(h2pre[i][:], m1a[:], prodAB[i][:, 0, :],
                                         start=True, stop=False)
                        nc.tensor.matmul(h2pre[i][:], m1b[:], prodAB[i][:, 1, :],
                                         start=False, stop=True)
                    h2 = {}
                    for i in range(G):
                        h2[i] = acts.tile([16, F], FP32R, tag="h2", name="h2")
                        nc.scalar.activation(h2[i][:], h2pre[i][:], relu)
                    h2rep = {}
                    for i in range(G):
                        h2rep[i] = ps_rep.tile([48, F], FP32, tag="rep", name="h2rep")
                        nc.tensor.matmul(h2rep[i][:], r3[:], h2[i][:],
                                         start=True, stop=True)
                    for i in range(G):
                        nc.vector.tensor_mul(pcm[0:48, sls[i]],
                                             _f(t2m[:, sls[i]]), h2rep[i][:])
                    y = {}
                    for i in range(G):
                        y[i] = ps_sm16.tile([16, F], FP32, tag="sm16", name="y")
                        nc.tensor.matmul(y[i][0:3, :], m2b[:], pcm[:, sls[i]],
                                         start=True, stop=True)
                    y_sb = outs.tile([3, FM], FP32, tag="ysb", name="ysb", bufs=1)
                    for i in range(G):
                        nc.scalar.copy(y_sb[:, sls[i]], y[i][0:3, :])
                    nc.gpsimd.dma_start(out=out[:, mp], in_=y_sb[:])

            if repeat == 1:
                body()
            else:
                with tc.For_i(0, repeat, 1):
                    body()

    nc.compile()
    return nc


_NC_CACHE: dict[int, object] = {}


def _get_nc(repeat: int = 1):
    if repeat not in _NC_CACHE:
        _NC_CACHE[repeat] = build_nc(repeat)
    return _NC_CACHE[repeat]


def make_in_maps(weight: np.ndarray, coor: np.ndarray) -> list[dict]:
    mats = _const_mats(coor)
    in_maps = []
    for k in range(N_CORES):
        b, hh = k // 2, k % 2
        shard = np.ascontiguousarray(
            weight[b, :, hh * 128:(hh + 1) * 128, :].reshape(N_CH, PIX),
            dtype=np.float32)
        in_maps.append({"w": shard, **mats})
    return in_maps


def assemble_out(results: list[dict]) -> np.ndarray:
    out = np.empty((B, 3, H, W), np.float32)
    for k in range(N_CORES):
        b, hh = k // 2, k % 2
        out[b, :, hh * 128:(hh + 1) * 128, :] = results[k]["out"].reshape(3, 128, W)
    return out


def kernel(input: np.ndarray, weight: np.ndarray, coor: np.ndarray) -> np.ndarray:
    nc = _get_nc(1)
    in_maps = make_in_maps(np.asarray(weight), np.asarray(coor))
    res = run_bass_kernel_spmd(nc, in_maps, core_ids=list(range(N_CORES)))
    return assemble_out(res.results)


# revision 20
# speedup vs baseline: 337.5542x; 1.0738x over previous
"""Trainium2 Bass kernel for the per-pixel locally-connected MLP (dense_mlp).

Reference computation (per batch b, pixel (h,w)):
    x0 = coor (2-vector, shared by all pixels)
    h1 = relu(W0 @ x0)        W0 = weight[b, 0:32].reshape(16, 2)   per pixel
    h2 = relu(W1 @ h1)        W1 = weight[b, 32:288].reshape(16,16) per pixel
    y  = W2 @ h2 + bias       W2 = weight[b, 288:336].reshape(3,16), bias = weight[b,336]
Output: [4, 3, 256, 256] float32.

Sharding: 8 cores, core k handles batch k//2, rows (k%2)*128:(k%2+1)*128
=> per-core weight shard [337, 32768] (channels x pixels), no cross-core comm.

On-chip layout: channels on SBUF partitions, pixels on the free axis.
Per-pixel matvecs = elementwise multiplies (VectorE) + partition-axis
reductions (TensorE matmuls against small 0/1 selection matrices built on the
host; `coor` is folded into the first matmul's stationary matrix).
"""

import sys

for _p in ("/opt/trn_rl_repo", "/root/.axon_site/_ro/trn_rl_repo"):
    if _p not in sys.path:
        sys.path.append(_p)

import numpy as np

import concourse.bass as bass
import concourse.tile as tile
from concourse import bacc, mybir
from concourse.bass_utils import run_bass_kernel_spmd

# ---------------------------------------------------------------- constants
B, H, W = 4, 256, 256
N_CH = 337            # 32 (L0) + 256 (L1) + 48 (L2) + 1 (bias)
N_CORES = 8
PIX = (B * H * W) // N_CORES  # 32768 pixels per core
F = 512               # pixels per compute chunk (one PSUM bank of fp32)
N_CHUNKS = PIX // F

FP32 = mybir.dt.float32
FP32R = mybir.dt.float32r


def _f(ap):
    return ap.bitcast(FP32)


def _const_mats(coor: np.ndarray) -> dict[str, np.ndarray]:
    """Small stationary matrices for the TensorE reductions."""
    cx, cy = float(coor[0]), float(coor[1])
    # even/odd stationaries write one pair-half each at base-0 dst APs:
    # cols 0:16 = even chunk, cols 32:48 = odd chunk, zeros elsewhere
    # (zeros accumulate as no-ops into the other half's partitions).
    s0 = np.zeros((2, 32, 48), np.float32)    # h1pre = S0.T @ w[0:32]
    for h in range(2):
        for i in range(16):
            s0[h, 2 * i, 32 * h + i] = cx
            s0[h, 2 * i + 1, 32 * h + i] = cy
    r8 = np.zeros((16, 128), np.float32)      # h1rep[m] = h1[m % 16]
    for m in range(128):
        r8[m % 16, m] = 1.0
    m1a = np.zeros((2, 128, 48), np.float32)  # h2pre[j] += sum_i prodA[16j+i]
    m1b = np.zeros((2, 128, 48), np.float32)
    for h in range(2):
        for k in range(128):
            m1a[h, k, 32 * h + k // 16] = 1.0
            m1b[h, k, 32 * h + 8 + k // 16] = 1.0
    # pair-fused layer-2: moving operand is h2pair [48,F] with chunk A's h2
    # at rows 0:16 and chunk B's at rows 32:48 (rows 16:32 are junk)
    r3_2 = np.zeros((48, 96), np.float32)     # h2rep2[k] = h2(k//48)[k % 16]
    for k in range(96):
        r3_2[(0 if k < 48 else 32) + k % 16, k] = 1.0
    m2b2 = np.zeros((98, 6), np.float32)      # y[h*3+j] = sum prodC + bias
    for k in range(96):
        m2b2[k, (k // 48) * 3 + (k % 48) // 16] = 1.0
    m2b2[96, 0:3] = 1.0                       # bias row, even chunk
    m2b2[97, 3:6] = 1.0                       # bias row, odd chunk
    return {"s0": s0, "r8": r8, "m1a": m1a, "m1b": m1b, "r3_2": r3_2,
            "m2b2": m2b2}


def build_nc(repeat: int = 1):
    """Build the per-core Bass program. `repeat` re-runs the whole kernel
    body sequentially (used only for differential HW timing)."""
    nc = bacc.Bacc(None, target_bir_lowering=False)

    w = nc.declare_dram_parameter("w", [N_CH, PIX], FP32R, isOutput=False)
    out = nc.declare_dram_parameter("out", [3, PIX], FP32, isOutput=True)
    c_s0 = nc.declare_dram_parameter("s0", [2, 32, 48], FP32R, isOutput=False)
    c_r8 = nc.declare_dram_parameter("r8", [16, 128], FP32R, isOutput=False)
    c_m1a = nc.declare_dram_parameter("m1a", [2, 128, 48], FP32R, isOutput=False)
    c_m1b = nc.declare_dram_parameter("m1b", [2, 128, 48], FP32R, isOutput=False)
    c_r3_2 = nc.declare_dram_parameter("r3_2", [48, 96], FP32R, isOutput=False)
    c_m2b2 = nc.declare_dram_parameter("m2b2", [98, 6], FP32R, isOutput=False)

    G = 8                      # chunks per software-pipeline group
    with tile.TileContext(nc) as tc:
        with (
            tc.tile_pool(name="consts", bufs=1) as consts,
            tc.tile_pool(name="loads", bufs=2) as loads,
            tc.tile_pool(name="acts", bufs=3) as acts,
            tc.tile_pool(name="prods", bufs=3) as prods,
            tc.tile_pool(name="outs", bufs=2) as outs,
            tc.tile_pool(name="ps_sm16", bufs=3, space="PSUM") as ps_sm16,
            tc.tile_pool(name="ps_h2p", bufs=2, space="PSUM") as ps_h2p,
            tc.tile_pool(name="ps_rep", bufs=3, space="PSUM") as ps_rep,
        ):
            s0 = consts.tile([32, 2, 48], FP32R)
            r8 = consts.tile([48, 128], FP32R)   # rows 0:16 and 32:48 both
                                                 # hold R8 (for base 0/32)
            m1a = consts.tile([128, 2, 48], FP32R)
            m1b = consts.tile([128, 2, 48], FP32R)
            r3_2 = consts.tile([48, 96], FP32R)
            m2b2 = consts.tile([98, 6], FP32R)
            for t, d in ((s0, c_s0.rearrange("h k m -> k h m")),
                         (r8[0:16, :], c_r8), (r8[32:48, :], c_r8),
                         (m1a, c_m1a.rearrange("h k m -> k h m")),
                         (m1b, c_m1b.rearrange("h k m -> k h m")),
                         (r3_2, c_r3_2), (m2b2, c_m2b2)):
                nc.sync.dma_start(out=t[:], in_=d[:])

            relu = mybir.ActivationFunctionType.Relu

            def body():
                # Macro-tile of G*F pixels; inside, sub-chunks are processed
                # in PAIRS sharing PSUM banks at partition offsets 0/32 (both
                # legal matmul base partitions). This halves ACT op count and
                # fuses all of layer 2 (rep, products, reduce+bias) per pair.
                FM = G * F
                NP = G // 2     # pairs per macro
                for g in range(N_CHUNKS // G):
                    mp = slice(g * FM, (g + 1) * FM)
                    sls = [slice(i * F, (i + 1) * F) for i in range(G)]
                    psl = [slice(p * F, (p + 1) * F) for p in range(NP)]

                    t0m = loads.tile([32, FM], FP32R, tag="t0", name="t0m")
                    t1m = loads.tile([128, 2, FM], FP32R, tag="t1", name="t1m")
                    # t2 pair-stacked: partition h*48+ch, free (pair, x);
                    # h = parity of the sub-chunk within its pair
                    t2m = loads.tile([96, NP, F], FP32R, tag="t2", name="t2m")
                    pcm = prods.tile([98, NP, F], FP32R, tag="pcm", name="pcm",
                                     bufs=2)
                    nc.sync.dma_start(out=t0m[:], in_=w[0:32, mp])
                    nc.sync.dma_start(
                        out=t1m[:],
                        in_=w[32:288, mp].rearrange("(b p) x -> p b x", b=2))
                    for h in range(2):
                        nc.sync.dma_start(
                            out=t2m[48 * h:48 * h + 48, :, :],
                            in_=bass.AP(tensor=w[:].tensor,
                                        offset=288 * PIX + g * FM + h * F,
                                        ap=[[PIX, 48], [2 * F, NP], [1, F]]))
                        nc.gpsimd.dma_start(
                            out=pcm[96 + h:97 + h, :, :],
                            in_=bass.AP(tensor=w[:].tensor,
                                        offset=336 * PIX + g * FM + h * F,
                                        ap=[[2 * F, NP], [1, F]]))

                    h1pre = {}
                    for p in range(NP):
                        h1pre[p] = ps_sm16.tile([48, F], FP32, tag="sm16",
                                                name="h1pre")
                        nc.tensor.matmul(h1pre[p][:], s0[:, 0, :],
                                         t0m[:, sls[2 * p]],
                                         start=True, stop=False)
                        nc.tensor.matmul(h1pre[p][:], s0[:, 1, :],
                                         t0m[:, sls[2 * p + 1]],
                                         start=False, stop=True)
                    h1 = {}
                    for p in range(NP):
                        h1[p] = acts.tile([48, F], FP32R, tag="h1", name="h1")
                        nc.scalar.activation(h1[p][:], h1pre[p][:], relu)
                    h1rep = {}
                    for p in range(NP):
                        for h in range(2):
                            h1rep[p, h] = ps_rep.tile([128, F], FP32,
                                                      tag="rep", name="h1rep")
                            nc.tensor.matmul(
                                h1rep[p, h][:],
                                r8[32 * h:32 * h + 16, :],
                                h1[p][32 * h:32 * h + 16, :],
                                start=True, stop=True)
                    prodAB = {}
                    for p in range(NP):
                        for h in range(2):
                            prodAB[p, h] = prods.tile([128, 2, F], FP32R,
                                                      tag="prodAB",
                                                      name="prodAB", bufs=3)
                            rp = h1rep[p, h]
                            rep2 = bass.AP(tensor=rp.tensor,
                                           offset=rp[:].offset,
                                           ap=[rp[:].ap[0], [0, 2],
                                               rp[:].ap[1]])
                            nc.vector.tensor_mul(
                                prodAB[p, h][:],
                                _f(t1m[:, :, sls[2 * p + h]]), rep2)
                    h2pre = {}
                    for p in range(NP):
                        h2pre[p] = ps_h2p.tile([48, F], FP32, tag="h2p",
                                               name="h2pre")
                        for h in range(2):
                            nc.tensor.matmul(h2pre[p][:],
                                             m1a[:, h, :], prodAB[p, h][:, 0, :],
                                             start=(h == 0), stop=False)
                            nc.tensor.matmul(h2pre[p][:],
                                             m1b[:, h, :], prodAB[p, h][:, 1, :],
                                             start=False, stop=(h == 1))
                    h2 = {}
                    for p in range(NP):
                        h2[p] = acts.tile([48, F], FP32R, tag="h2", name="h2")
                        nc.scalar.activation(h2[p][:], h2pre[p][:], relu)
                    h2rep = {}
                    for p in range(NP):
                        h2rep[p] = ps_rep.tile([96, F], FP32, tag="rep",
                                               name="h2rep")
                        nc.tensor.matmul(h2rep[p][:], r3_2[:], h2[p][:],
                                         start=True, stop=True)
                    for p in range(NP):
                        nc.vector.tensor_mul(pcm[0:96, p, :],
                                             _f(t2m[:, p, :]), h2rep[p][:])
                    y = {}
                    for p in range(NP):
                        y[p] = ps_sm16.tile([6, F], FP32, tag="sm16", name="y")
                        nc.tensor.matmul(y[p][:], m2b2[:], pcm[:, p, :],
                                         start=True, stop=True)
                    y_sb = outs.tile([6, NP, F], FP32, tag="ysb", name="ysb",
                                     bufs=2)
                    for p in range(NP):
                        nc.scalar.copy(y_sb[:, p, :], y[p][:])
                    # y_sb partition h*3+j, free (pair, x) ->
                    # out[j, g*FM + pair*2F + h*F + x]
                    for h in range(2):
                        nc.gpsimd.dma_start(
                            out=bass.AP(tensor=out[:].tensor,
                                        offset=g * FM + h * F,
                                        ap=[[PIX, 3], [2 * F, NP], [1, F]]),
                            in_=y_sb[3 * h:3 * h + 3, :, :])

            if repeat == 1:
                body()
            else:
                with tc.For_i(0, repeat, 1):
                    body()

    nc.compile()
    return nc


_NC_CACHE: dict[int, object] = {}


def _get_nc(repeat: int = 1):
    if repeat not in _NC_CACHE:
        _NC_CACHE[repeat] = build_nc(repeat)
    return _NC_CACHE[repeat]


def make_in_maps(weight: np.ndarray, coor: np.ndarray) -> list[dict]:
    mats = _const_mats(coor)
    in_maps = []
    for k in range(N_CORES):
        b, hh = k // 2, k % 2
        shard = np.ascontiguousarray(
            weight[b, :, hh * 128:(hh + 1) * 128, :].reshape(N_CH, PIX),
            dtype=np.float32)
        in_maps.append({"w": shard, **mats})
    return in_maps


def assemble_out(results: list[dict]) -> np.ndarray:
    out = np.empty((B, 3, H, W), np.float32)
    for k in range(N_CORES):
        b, hh = k // 2, k % 2
        out[b, :, hh * 128:(hh + 1) * 128, :] = results[k]["out"].reshape(3, 128, W)
    return out


def kernel(input: np.ndarray, weight: np.ndarray, coor: np.ndarray) -> np.ndarray:
    nc = _get_nc(1)
    in_maps = make_in_maps(np.asarray(weight), np.asarray(coor))
    res = run_bass_kernel_spmd(nc, in_maps, core_ids=list(range(N_CORES)))
    return assemble_out(res.results)


# revision 21
# speedup vs baseline: 396.9254x; 1.1759x over previous
"""Trainium2 Bass kernel for the per-pixel locally-connected MLP (dense_mlp).

Reference computation (per batch b, pixel (h,w)):
    x0 = coor (2-vector, shared by all pixels)
    h1 = relu(W0 @ x0)        W0 = weight[b, 0:32].reshape(16, 2)   per pixel
    h2 = relu(W1 @ h1)        W1 = weight[b, 32:288].reshape(16,16) per pixel
    y  = W2 @ h2 + bias       W2 = weight[b, 288:336].reshape(3,16), bias = weight[b,336]
Output: [4, 3, 256, 256] float32.

Sharding: 8 cores, core k handles batch k//2, rows (k%2)*128:(k%2+1)*128
=> per-core weight shard [337, 32768] (channels x pixels), no cross-core comm.

On-chip layout: channels on SBUF partitions, pixels on the free axis.
Per-pixel matvecs = elementwise multiplies (VectorE) + partition-axis
reductions (TensorE matmuls against small 0/1 selection matrices built on the
host; `coor` is folded into the first matmul's stationary matrix).
"""

import sys

for _p in ("/opt/trn_rl_repo", "/root/.axon_site/_ro/trn_rl_repo"):
    if _p not in sys.path:
        sys.path.append(_p)

import numpy as np

import concourse.bass as bass
import concourse.tile as tile
from concourse import bacc, mybir
from concourse.bass_utils import run_bass_kernel_spmd

# ---------------------------------------------------------------- constants
B, H, W = 4, 256, 256
N_CH = 337            # 32 (L0) + 256 (L1) + 48 (L2) + 1 (bias)
N_CORES = 8
PIX = (B * H * W) // N_CORES  # 32768 pixels per core
F = 512               # pixels per compute chunk (one PSUM bank of fp32)
N_CHUNKS = PIX // F

FP32 = mybir.dt.float32
FP32R = mybir.dt.float32r
FP16 = mybir.dt.float16


def _f(ap):
    return ap.bitcast(FP32)


def _const_mats(coor: np.ndarray) -> dict[str, np.ndarray]:
    """Small stationary matrices for the TensorE reductions."""
    cx, cy = float(coor[0]), float(coor[1])
    # even/odd stationaries write one pair-half each at base-0 dst APs:
    # cols 0:16 = even chunk, cols 32:48 = odd chunk, zeros elsewhere
    # (zeros accumulate as no-ops into the other half's partitions).
    s0 = np.zeros((2, 32, 48), np.float32)    # h1pre = S0.T @ w[0:32]
    for h in range(2):
        for i in range(16):
            s0[h, 2 * i, 32 * h + i] = cx
            s0[h, 2 * i + 1, 32 * h + i] = cy
    r8 = np.zeros((16, 128), np.float32)      # h1rep[m] = h1[m % 16]
    for m in range(128):
        r8[m % 16, m] = 1.0
    m1a = np.zeros((2, 128, 48), np.float32)  # h2pre[j] += sum_i prodA[16j+i]
    m1b = np.zeros((2, 128, 48), np.float32)
    for h in range(2):
        for k in range(128):
            m1a[h, k, 32 * h + k // 16] = 1.0
            m1b[h, k, 32 * h + 8 + k // 16] = 1.0
    # pair-fused layer-2: moving operand is h2pair [48,F] with chunk A's h2
    # at rows 0:16 and chunk B's at rows 32:48 (rows 16:32 are junk)
    r3_2 = np.zeros((48, 96), np.float32)     # h2rep2[k] = h2(k//48)[k % 16]
    for k in range(96):
        r3_2[(0 if k < 48 else 32) + k % 16, k] = 1.0
    m2b2 = np.zeros((98, 6), np.float32)      # y[h*3+j] = sum prodC + bias
    for k in range(96):
        m2b2[k, (k // 48) * 3 + (k % 48) // 16] = 1.0
    m2b2[96, 0:3] = 1.0                       # bias row, even chunk
    m2b2[97, 3:6] = 1.0                       # bias row, odd chunk
    return {"s0": s0.astype(np.float16), "r8": r8, "m1a": m1a, "m1b": m1b,
            "r3_2": r3_2, "m2b2": m2b2}


def build_nc(repeat: int = 1):
    """Build the per-core Bass program. `repeat` re-runs the whole kernel
    body sequentially (used only for differential HW timing)."""
    nc = bacc.Bacc(None, target_bir_lowering=False)

    w = nc.declare_dram_parameter("w", [N_CH, PIX], FP16, isOutput=False)
    out = nc.declare_dram_parameter("out", [3, PIX], FP32, isOutput=True)
    c_s0 = nc.declare_dram_parameter("s0", [2, 32, 48], FP16, isOutput=False)
    c_r8 = nc.declare_dram_parameter("r8", [16, 128], FP32R, isOutput=False)
    c_m1a = nc.declare_dram_parameter("m1a", [2, 128, 48], FP32R, isOutput=False)
    c_m1b = nc.declare_dram_parameter("m1b", [2, 128, 48], FP32R, isOutput=False)
    c_r3_2 = nc.declare_dram_parameter("r3_2", [48, 96], FP32R, isOutput=False)
    c_m2b2 = nc.declare_dram_parameter("m2b2", [98, 6], FP32R, isOutput=False)

    G = 8                      # chunks per software-pipeline group
    with tile.TileContext(nc) as tc:
        with (
            tc.tile_pool(name="consts", bufs=1) as consts,
            tc.tile_pool(name="loads", bufs=2) as loads,
            tc.tile_pool(name="acts", bufs=3) as acts,
            tc.tile_pool(name="prods", bufs=3) as prods,
            tc.tile_pool(name="outs", bufs=2) as outs,
            tc.tile_pool(name="ps_sm16", bufs=3, space="PSUM") as ps_sm16,
            tc.tile_pool(name="ps_h2p", bufs=2, space="PSUM") as ps_h2p,
            tc.tile_pool(name="ps_rep", bufs=3, space="PSUM") as ps_rep,
        ):
            s0 = consts.tile([32, 2, 48], FP16)
            r8 = consts.tile([48, 128], FP32R)   # rows 0:16 and 32:48 both
                                                 # hold R8 (for base 0/32)
            m1a = consts.tile([128, 2, 48], FP32R)
            m1b = consts.tile([128, 2, 48], FP32R)
            r3_2 = consts.tile([48, 96], FP32R)
            m2b2 = consts.tile([98, 6], FP32R)
            for t, d in ((s0, c_s0.rearrange("h k m -> k h m")),
                         (r8[0:16, :], c_r8), (r8[32:48, :], c_r8),
                         (m1a, c_m1a.rearrange("h k m -> k h m")),
                         (m1b, c_m1b.rearrange("h k m -> k h m")),
                         (r3_2, c_r3_2), (m2b2, c_m2b2)):
                nc.sync.dma_start(out=t[:], in_=d[:])

            relu = mybir.ActivationFunctionType.Relu

            def body():
                # Macro-tile of G*F pixels; inside, sub-chunks are processed
                # in PAIRS sharing PSUM banks at partition offsets 0/32 (both
                # legal matmul base partitions). This halves ACT op count and
                # fuses all of layer 2 (rep, products, reduce+bias) per pair.
                FM = G * F
                NP = G // 2     # pairs per macro
                for g in range(N_CHUNKS // G):
                    mp = slice(g * FM, (g + 1) * FM)
                    sls = [slice(i * F, (i + 1) * F) for i in range(G)]
                    psl = [slice(p * F, (p + 1) * F) for p in range(NP)]

                    t0m = loads.tile([32, FM], FP16, tag="t0", name="t0m")
                    t1m = loads.tile([128, 2, FM], FP16, tag="t1", name="t1m")
                    # t2 pair-stacked: partition h*48+ch, free (pair, x);
                    # h = parity of the sub-chunk within its pair
                    t2m = loads.tile([96, NP, F], FP16, tag="t2", name="t2m")
                    pcm = prods.tile([98, NP, F], FP32R, tag="pcm", name="pcm",
                                     bufs=2)
                    nc.sync.dma_start(out=t0m[:], in_=w[0:32, mp])
                    nc.sync.dma_start(
                        out=t1m[:],
                        in_=w[32:288, mp].rearrange("(b p) x -> p b x", b=2))
                    for h in range(2):
                        nc.sync.dma_start(
                            out=t2m[48 * h:48 * h + 48, :, :],
                            in_=bass.AP(tensor=w[:].tensor,
                                        offset=288 * PIX + g * FM + h * F,
                                        ap=[[PIX, 48], [2 * F, NP], [1, F]]))
                        nc.gpsimd.dma_start(
                            out=pcm[96 + h:97 + h, :, :],
                            in_=bass.AP(tensor=w[:].tensor,
                                        offset=336 * PIX + g * FM + h * F,
                                        ap=[[2 * F, NP], [1, F]]))

                    h1pre = {}
                    for p in range(NP):
                        h1pre[p] = ps_sm16.tile([48, F], FP32, tag="sm16",
                                                name="h1pre")
                        nc.tensor.matmul(h1pre[p][:], s0[:, 0, :],
                                         t0m[:, sls[2 * p]],
                                         start=True, stop=False)
                        nc.tensor.matmul(h1pre[p][:], s0[:, 1, :],
                                         t0m[:, sls[2 * p + 1]],
                                         start=False, stop=True)
                    h1 = {}
                    for p in range(NP):
                        h1[p] = acts.tile([48, F], FP32R, tag="h1", name="h1")
                        nc.scalar.activation(h1[p][:], h1pre[p][:], relu)
                    h1rep = {}
                    for p in range(NP):
                        for h in range(2):
                            h1rep[p, h] = ps_rep.tile([128, F], FP32,
                                                      tag="rep", name="h1rep")
                            nc.tensor.matmul(
                                h1rep[p, h][:],
                                r8[32 * h:32 * h + 16, :],
                                h1[p][32 * h:32 * h + 16, :],
                                start=True, stop=True)
                    prodAB = {}
                    for p in range(NP):
                        for h in range(2):
                            prodAB[p, h] = prods.tile([128, 2, F], FP32R,
                                                      tag="prodAB",
                                                      name="prodAB", bufs=3)
                            rp = h1rep[p, h]
                            rep2 = bass.AP(tensor=rp.tensor,
                                           offset=rp[:].offset,
                                           ap=[rp[:].ap[0], [0, 2],
                                               rp[:].ap[1]])
                            nc.vector.tensor_mul(
                                prodAB[p, h][:],
                                t1m[:, :, sls[2 * p + h]], rep2)
                    h2pre = {}
                    for p in range(NP):
                        h2pre[p] = ps_h2p.tile([48, F], FP32, tag="h2p",
                                               name="h2pre")
                        for h in range(2):
                            nc.tensor.matmul(h2pre[p][:],
                                             m1a[:, h, :], prodAB[p, h][:, 0, :],
                                             start=(h == 0), stop=False)
                            nc.tensor.matmul(h2pre[p][:],
                                             m1b[:, h, :], prodAB[p, h][:, 1, :],
                                             start=False, stop=(h == 1))
                    h2 = {}
                    for p in range(NP):
                        h2[p] = acts.tile([48, F], FP32R, tag="h2", name="h2")
                        nc.scalar.activation(h2[p][:], h2pre[p][:], relu)
                    h2rep = {}
                    for p in range(NP):
                        h2rep[p] = ps_rep.tile([96, F], FP32, tag="rep",
                                               name="h2rep")
                        nc.tensor.matmul(h2rep[p][:], r3_2[:], h2[p][:],
                                         start=True, stop=True)
                    for p in range(NP):
                        nc.vector.tensor_mul(pcm[0:96, p, :],
                                             t2m[:, p, :], h2rep[p][:])
                    y = {}
                    for p in range(NP):
                        y[p] = ps_sm16.tile([6, F], FP32, tag="sm16", name="y")
                        nc.tensor.matmul(y[p][:], m2b2[:], pcm[:, p, :],
                                         start=True, stop=True)
                    y_sb = outs.tile([6, NP, F], FP32, tag="ysb", name="ysb",
                                     bufs=2)
                    for p in range(NP):
                        nc.scalar.copy(y_sb[:, p, :], y[p][:])
                    # y_sb partition h*3+j, free (pair, x) ->
                    # out[j, g*FM + pair*2F + h*F + x]
                    for h in range(2):
                        nc.gpsimd.dma_start(
                            out=bass.AP(tensor=out[:].tensor,
                                        offset=g * FM + h * F,
                                        ap=[[PIX, 3], [2 * F, NP], [1, F]]),
                            in_=y_sb[3 * h:3 * h + 3, :, :])

            if repeat == 1:
                body()
            else:
                with tc.For_i(0, repeat, 1):
                    body()

    nc.compile()
    return nc


_NC_CACHE: dict[int, object] = {}


def _get_nc(repeat: int = 1):
    if repeat not in _NC_CACHE:
        _NC_CACHE[repeat] = build_nc(repeat)
    return _NC_CACHE[repeat]


def make_in_maps(weight: np.ndarray, coor: np.ndarray) -> list[dict]:
    mats = _const_mats(coor)
    in_maps = []
    for k in range(N_CORES):
        b, hh = k // 2, k % 2
        shard = np.ascontiguousarray(
            weight[b, :, hh * 128:(hh + 1) * 128, :].reshape(N_CH, PIX),
            dtype=np.float16)
        in_maps.append({"w": shard, **mats})
    return in_maps


def assemble_out(results: list[dict]) -> np.ndarray:
    out = np.empty((B, 3, H, W), np.float32)
    for k in range(N_CORES):
        b, hh = k // 2, k % 2
        out[b, :, hh * 128:(hh + 1) * 128, :] = results[k]["out"].reshape(3, 128, W)
    return out


def kernel(input: np.ndarray, weight: np.ndarray, coor: np.ndarray) -> np.ndarray:
    nc = _get_nc(1)
    in_maps = make_in_maps(np.asarray(weight), np.asarray(coor))
    res = run_bass_kernel_spmd(nc, in_maps, core_ids=list(range(N_CORES)))
    return assemble_out(res.results)


# revision 23
# speedup vs baseline: 460.2707x; 1.1596x over previous
"""Trainium2 Bass kernel for the per-pixel locally-connected MLP (dense_mlp).

Reference computation (per batch b, pixel (h,w)):
    x0 = coor (2-vector, shared by all pixels)
    h1 = relu(W0 @ x0)        W0 = weight[b, 0:32].reshape(16, 2)   per pixel
    h2 = relu(W1 @ h1)        W1 = weight[b, 32:288].reshape(16,16) per pixel
    y  = W2 @ h2 + bias       W2 = weight[b, 288:336].reshape(3,16), bias = weight[b,336]
Output: [4, 3, 256, 256] float32.

Sharding: 8 cores, core k handles batch k//2, rows (k%2)*128:(k%2+1)*128
=> per-core weight shard [337, 32768] (channels x pixels), no cross-core comm.

On-chip layout: channels on SBUF partitions, pixels on the free axis.
Per-pixel matvecs = elementwise multiplies (VectorE) + partition-axis
reductions (TensorE matmuls against small 0/1 selection matrices built on the
host; `coor` is folded into the first matmul's stationary matrix).
"""

import sys

for _p in ("/opt/trn_rl_repo", "/root/.axon_site/_ro/trn_rl_repo"):
    if _p not in sys.path:
        sys.path.append(_p)

import numpy as np

import concourse.bass as bass
import concourse.tile as tile
from concourse import bacc, mybir
from concourse.bass_utils import run_bass_kernel_spmd

# ---------------------------------------------------------------- constants
B, H, W = 4, 256, 256
N_CH = 337            # 32 (L0) + 256 (L1) + 48 (L2) + 1 (bias)
N_CORES = 8
PIX = (B * H * W) // N_CORES  # 32768 pixels per core
F = 512               # pixels per compute chunk (one PSUM bank of fp32)
N_CHUNKS = PIX // F

FP32 = mybir.dt.float32
FP32R = mybir.dt.float32r
FP16 = mybir.dt.float16


def _f(ap):
    return ap.bitcast(FP32)


def _const_mats(coor: np.ndarray) -> dict[str, np.ndarray]:
    """Small stationary matrices for the TensorE reductions."""
    cx, cy = float(coor[0]), float(coor[1])
    # even/odd stationaries write one pair-half each at base-0 dst APs:
    # cols 0:16 = even chunk, cols 32:48 = odd chunk, zeros elsewhere
    # (zeros accumulate as no-ops into the other half's partitions).
    s0 = np.zeros((2, 32, 48), np.float32)    # h1pre = S0.T @ w[0:32]
    for h in range(2):
        for i in range(16):
            s0[h, 2 * i, 32 * h + i] = cx
            s0[h, 2 * i + 1, 32 * h + i] = cy
    r8 = np.zeros((16, 128), np.float32)      # h1rep[m] = h1[m % 16]
    for m in range(128):
        r8[m % 16, m] = 1.0
    m1a = np.zeros((2, 128, 48), np.float32)  # h2pre[j] += sum_i prodA[16j+i]
    m1b = np.zeros((2, 128, 48), np.float32)
    for h in range(2):
        for k in range(128):
            m1a[h, k, 32 * h + k // 16] = 1.0
            m1b[h, k, 32 * h + 8 + k // 16] = 1.0
    # pair-fused layer-2: moving operand is h2pair [48,F] with chunk A's h2
    # at rows 0:16 and chunk B's at rows 32:48 (rows 16:32 are junk)
    r3_2 = np.zeros((48, 96), np.float32)     # h2rep2[k] = h2(k//48)[k % 16]
    for k in range(96):
        r3_2[(0 if k < 48 else 32) + k % 16, k] = 1.0
    m2b2 = np.zeros((98, 6), np.float32)      # y[h*3+j] = sum prodC + bias
    for k in range(96):
        m2b2[k, (k // 48) * 3 + (k % 48) // 16] = 1.0
    m2b2[96, 0:3] = 1.0                       # bias row, even chunk
    m2b2[97, 3:6] = 1.0                       # bias row, odd chunk
    return {"s0": s0.astype(np.float16), "r8": r8, "m1a": m1a, "m1b": m1b,
            "r3_2": r3_2, "m2b2": m2b2}


def build_nc(repeat: int = 1):
    """Build the per-core Bass program. `repeat` re-runs the whole kernel
    body sequentially (used only for differential HW timing)."""
    nc = bacc.Bacc(None, target_bir_lowering=False)

    w = nc.declare_dram_parameter("w", [N_CH, PIX], FP16, isOutput=False)
    out = nc.declare_dram_parameter("out", [3, PIX], FP32, isOutput=True)
    c_s0 = nc.declare_dram_parameter("s0", [2, 32, 48], FP16, isOutput=False)
    c_r8 = nc.declare_dram_parameter("r8", [16, 128], FP32R, isOutput=False)
    c_m1a = nc.declare_dram_parameter("m1a", [2, 128, 48], FP32R, isOutput=False)
    c_m1b = nc.declare_dram_parameter("m1b", [2, 128, 48], FP32R, isOutput=False)
    c_r3_2 = nc.declare_dram_parameter("r3_2", [48, 96], FP32R, isOutput=False)
    c_m2b2 = nc.declare_dram_parameter("m2b2", [98, 6], FP32R, isOutput=False)

    G = 8                      # chunks per software-pipeline group
    with tile.TileContext(nc) as tc:
        with (
            tc.tile_pool(name="consts", bufs=1) as consts,
            tc.tile_pool(name="loads", bufs=2) as loads,
            tc.tile_pool(name="acts", bufs=4) as acts,
            tc.tile_pool(name="prods", bufs=3) as prods,
            tc.tile_pool(name="outs", bufs=2) as outs,
            tc.tile_pool(name="ps_sm16", bufs=3, space="PSUM") as ps_sm16,
            tc.tile_pool(name="ps_h2p", bufs=2, space="PSUM") as ps_h2p,
            tc.tile_pool(name="ps_rep", bufs=3, space="PSUM") as ps_rep,
        ):
            s0 = consts.tile([32, 2, 48], FP16)
            r8 = consts.tile([48, 128], FP32R)   # rows 0:16 and 32:48 both
                                                 # hold R8 (for base 0/32)
            m1a = consts.tile([128, 2, 48], FP32R)
            m1b = consts.tile([128, 2, 48], FP32R)
            r3_2 = consts.tile([48, 96], FP32R)
            m2b2 = consts.tile([98, 6], FP32R)
            for t, d in ((s0, c_s0.rearrange("h k m -> k h m")),
                         (r8[0:16, :], c_r8), (r8[32:48, :], c_r8),
                         (m1a, c_m1a.rearrange("h k m -> k h m")),
                         (m1b, c_m1b.rearrange("h k m -> k h m")),
                         (r3_2, c_r3_2), (m2b2, c_m2b2)):
                nc.sync.dma_start(out=t[:], in_=d[:])

            relu = mybir.ActivationFunctionType.Relu

            def body():
                # Macro-tile of G*F pixels; inside, sub-chunks are processed
                # in PAIRS sharing PSUM banks at partition offsets 0/32 (both
                # legal matmul base partitions). This halves ACT op count and
                # fuses all of layer 2 (rep, products, reduce+bias) per pair.
                FM = G * F
                NP = G // 2     # pairs per macro
                for g in range(N_CHUNKS // G):
                    mp = slice(g * FM, (g + 1) * FM)
                    sls = [slice(i * F, (i + 1) * F) for i in range(G)]
                    psl = [slice(p * F, (p + 1) * F) for p in range(NP)]

                    t0m = loads.tile([32, FM], FP16, tag="t0", name="t0m", bufs=3)
                    t1m = loads.tile([128, 2, FM], FP16, tag="t1", name="t1m", bufs=3)
                    # t2 pair-stacked: partition h*48+ch, free (pair, x);
                    # h = parity of the sub-chunk within its pair
                    t2m = loads.tile([96, NP, F], FP16, tag="t2", name="t2m", bufs=3)
                    pcm = prods.tile([98, NP, F], FP32R, tag="pcm", name="pcm",
                                     bufs=3)
                    nc.sync.dma_start(out=t0m[:], in_=w[0:32, mp])
                    nc.sync.dma_start(
                        out=t1m[:],
                        in_=w[32:288, mp].rearrange("(b p) x -> p b x", b=2))
                    for h in range(2):
                        nc.sync.dma_start(
                            out=t2m[48 * h:48 * h + 48, :, :],
                            in_=bass.AP(tensor=w[:].tensor,
                                        offset=288 * PIX + g * FM + h * F,
                                        ap=[[PIX, 48], [2 * F, NP], [1, F]]))
                        nc.gpsimd.dma_start(
                            out=pcm[96 + h:97 + h, :, :],
                            in_=bass.AP(tensor=w[:].tensor,
                                        offset=336 * PIX + g * FM + h * F,
                                        ap=[[2 * F, NP], [1, F]]))

                    h1pre = {}
                    for p in range(NP):
                        h1pre[p] = ps_sm16.tile([48, F], FP32, tag="sm16",
                                                name="h1pre")
                        nc.tensor.matmul(h1pre[p][:], s0[:, 0, :],
                                         t0m[:, sls[2 * p]],
                                         start=True, stop=False)
                        nc.tensor.matmul(h1pre[p][:], s0[:, 1, :],
                                         t0m[:, sls[2 * p + 1]],
                                         start=False, stop=True)
                    h1 = {}
                    for p in range(NP):
                        h1[p] = acts.tile([48, F], FP32R, tag="h1", name="h1")
                        nc.scalar.activation(h1[p][:], h1pre[p][:], relu)
                    h1rep = {}
                    for p in range(NP):
                        for h in range(2):
                            h1rep[p, h] = ps_rep.tile([128, F], FP32,
                                                      tag="rep", name="h1rep")
                            nc.tensor.matmul(
                                h1rep[p, h][:],
                                r8[32 * h:32 * h + 16, :],
                                h1[p][32 * h:32 * h + 16, :],
                                start=True, stop=True)
                    prodAB = {}
                    for p in range(NP):
                        for h in range(2):
                            prodAB[p, h] = prods.tile([128, 2, F], FP32R,
                                                      tag="prodAB",
                                                      name="prodAB", bufs=3)
                            rp = h1rep[p, h]
                            rep2 = bass.AP(tensor=rp.tensor,
                                           offset=rp[:].offset,
                                           ap=[rp[:].ap[0], [0, 2],
                                               rp[:].ap[1]])
                            nc.vector.tensor_mul(
                                prodAB[p, h][:],
                                t1m[:, :, sls[2 * p + h]], rep2)
                    h2pre = {}
                    for p in range(NP):
                        h2pre[p] = ps_h2p.tile([48, F], FP32, tag="h2p",
                                               name="h2pre")
                        for h in range(2):
                            nc.tensor.matmul(h2pre[p][:],
                                             m1a[:, h, :], prodAB[p, h][:, 0, :],
                                             start=(h == 0), stop=False)
                            nc.tensor.matmul(h2pre[p][:],
                                             m1b[:, h, :], prodAB[p, h][:, 1, :],
                                             start=False, stop=(h == 1))
                    h2 = {}
                    for p in range(NP):
                        h2[p] = acts.tile([48, F], FP32R, tag="h2", name="h2")
                        nc.scalar.activation(h2[p][:], h2pre[p][:], relu)
                    h2rep = {}
                    for p in range(NP):
                        h2rep[p] = ps_rep.tile([96, F], FP32, tag="rep",
                                               name="h2rep")
                        nc.tensor.matmul(h2rep[p][:], r3_2[:], h2[p][:],
                                         start=True, stop=True)
                    for p in range(NP):
                        nc.vector.tensor_mul(pcm[0:96, p, :],
                                             t2m[:, p, :], h2rep[p][:])
                    y = {}
                    for p in range(NP):
                        y[p] = ps_sm16.tile([6, F], FP32, tag="sm16", name="y")
                        nc.tensor.matmul(y[p][:], m2b2[:], pcm[:, p, :],
                                         start=True, stop=True)
                    y_sb = outs.tile([6, NP, F], FP32, tag="ysb", name="ysb",
                                     bufs=2)
                    for p in range(NP):
                        nc.scalar.copy(y_sb[:, p, :], y[p][:])
                    # y_sb partition h*3+j, free (pair, x) ->
                    # out[j, g*FM + pair*2F + h*F + x]
                    for h in range(2):
                        nc.gpsimd.dma_start(
                            out=bass.AP(tensor=out[:].tensor,
                                        offset=g * FM + h * F,
                                        ap=[[PIX, 3], [2 * F, NP], [1, F]]),
                            in_=y_sb[3 * h:3 * h + 3, :, :])

            if repeat == 1:
                body()
            else:
                with tc.For_i(0, repeat, 1):
                    body()

    nc.compile()
    return nc


_NC_CACHE: dict[int, object] = {}


def _get_nc(repeat: int = 1):
    if repeat not in _NC_CACHE:
        _NC_CACHE[repeat] = build_nc(repeat)
    return _NC_CACHE[repeat]


def make_in_maps(weight: np.ndarray, coor: np.ndarray) -> list[dict]:
    mats = _const_mats(coor)
    in_maps = []
    for k in range(N_CORES):
        b, hh = k // 2, k % 2
        shard = np.ascontiguousarray(
            weight[b, :, hh * 128:(hh + 1) * 128, :].reshape(N_CH, PIX),
            dtype=np.float16)
        in_maps.append({"w": shard, **mats})
    return in_maps


def assemble_out(results: list[dict]) -> np.ndarray:
    out = np.empty((B, 3, H, W), np.float32)
    for k in range(N_CORES):
        b, hh = k // 2, k % 2
        out[b, :, hh * 128:(hh + 1) * 128, :] = results[k]["out"].reshape(3, 128, W)
    return out


def kernel(input: np.ndarray, weight: np.ndarray, coor: np.ndarray) -> np.ndarray:
    nc = _get_nc(1)
    in_maps = make_in_maps(np.asarray(weight), np.asarray(coor))
    res = run_bass_kernel_spmd(nc, in_maps, core_ids=list(range(N_CORES)))
    return assemble_out(res.results)
